# revision 9
# baseline (speedup 1.0000x reference)
"""AGAT layer (GNN message passing) on 8 TRN2 NeuronCores.

Strategy v2 (dst-sharded, padded-CSR, single collective, lean DVE):
  - Nodes degree-sorted into 128-node tiles; tile groups of 8 dealt
    round-robin to cores (identical SPMD schedule, balanced edges).
  - Each core computes z = h @ W_fc.T for its 12544-node block on PE,
    packs 4 nodes per 256B row with (d,q)-INTERLEAVED layout
    (row col = d*4+q), AllGathers the table.
  - Per tile, 256B rows are fetched with dma_gather (int16 row idx);
    the 1-of-4 node select = qm mask-mult + 2-level contiguous add tree.
  - Attention: a_src dot on DVE, a_dst via per-tile PE matmul,
    leaky-relu + exp + denominator accumulation on the Scalar engine.
  - dst nodes on partitions, edge slots on the free dim; each core owns
    its dst nodes -> no accumulator all-reduce.
"""
import os
import numpy as np

LAST_EXEC_NS = None


def _install_ntff_shim():
    """Register the NTFF profile hook bass_utils expects under axon."""
    import sys
    import types
    import antenv
    if "antenv.axon_hooks" in sys.modules:
        return
    mod = types.ModuleType("antenv.axon_hooks")
    mod._hook = None
    mod.set_axon_ntff_profile_hook = lambda h: setattr(mod, "_hook", h)
    mod.get_axon_ntff_profile_hook = lambda: mod._hook
    sys.modules["antenv.axon_hooks"] = mod
    antenv.axon_hooks = mod
    try:
        from trn_agent_boot.trn_boot import _ntff_profile_via_ctypes
        mod.set_axon_ntff_profile_hook(
            _ntff_profile_via_ctypes("/opt/axon/libaxon_pjrt.so"))
    except Exception:
        pass


N_NODES = 100000
N_EDGES = 3200000
IN_DIM = 62
OUT_DIM = 16
EDGE_DIM = 2
N_CORES = 8
TILE = 128
TPC = 98                      # tiles per core
NT = N_CORES * TPC            # 784 tiles
NPAD = NT * TILE              # 100352 padded nodes
BLOCK = TPC * TILE            # 12544 nodes per core block
TROWS = NPAD // 4             # 25088 packed table rows
ROWW = 64                     # table row width in f32 (4 nodes x 16)
MAXC = 8                      # max k-columns per gather call -> NI <= 1024
NEG_BIAS = -6000.0
GRP = 4                       # tiles per z-matmul group (free dim 512)
NGRP = (TPC + GRP - 1) // GRP


def _host_prep(h, e, src, dst):
    deg = np.bincount(dst, minlength=N_NODES)
    order = np.argsort(-deg, kind="stable").astype(np.int64)
    order_pad = np.concatenate([order, np.full(NPAD - N_NODES, -1, np.int64)])

    degp = np.concatenate([deg, np.zeros(NPAD - N_NODES, np.int64)])
    deg_of = np.where(order_pad >= 0, degp[np.maximum(order_pad, 0)], 0)
    Kg = deg_of.reshape(NT, TILE).max(axis=1)
    K_sched = np.maximum(Kg.reshape(TPC, N_CORES).max(axis=1), 1).astype(np.int64)

    # table layout: core c block rows [c*BLOCK, (c+1)*BLOCK); tile i of core c
    # = global tile 8i+c at rows c*BLOCK + i*TILE + p
    node_at = np.full((N_CORES, BLOCK), -1, np.int64)
    tabpos = np.full(N_NODES, -1, np.int64)
    for g in range(NT):
        i, c = divmod(g, N_CORES)
        nodes = order_pad[g * TILE:(g + 1) * TILE]
        node_at[c, i * TILE:(i + 1) * TILE] = nodes
        valid = nodes >= 0
        tp = c * BLOCK + i * TILE + np.nonzero(valid)[0]
        tabpos[nodes[valid]] = tp

    e_order = np.argsort(dst, kind="stable")
    csr_off = np.zeros(N_NODES + 1, np.int64)
    np.cumsum(deg, out=csr_off[1:])

    Ksum = int(K_sched.sum())
    col_off = np.zeros(TPC + 1, np.int64)
    np.cumsum(K_sched, out=col_off[1:])

    idx16 = np.zeros((N_CORES, 128, 16 * Ksum), np.int16)
    blob = np.zeros((N_CORES, 128, 9 * Ksum), np.float32)
    hsh = np.zeros((N_CORES, IN_DIM, BLOCK), np.float32)

    src_pos = tabpos[src]
    qrow_all = (src_pos // 4).astype(np.int32)
    qsel_all = (src_pos % 4).astype(np.int64)

    for c in range(N_CORES):
        hrows = node_at[c]
        hv = hrows >= 0
        hsh[c][:, hv] = h[hrows[hv]].T
        for i in range(TPC):
            K = int(K_sched[i])
            nodes = node_at[c, i * TILE:(i + 1) * TILE]
            co = int(col_off[i])
            eb = np.full((TILE, K), -1, np.int64)
            for p in range(TILE):
                n = nodes[p]
                if n < 0:
                    continue
                eds = e_order[csr_off[n]:csr_off[n + 1]]
                eb[p, :len(eds)] = eds
            vm = eb >= 0
            ebs = np.maximum(eb, 0)
            qi = np.where(vm, qrow_all[ebs], 0).astype(np.int16)  # [128, K]
            k0 = 0
            while k0 < K:
                Kc = min(MAXC, K - k0)
                NI = Kc * TILE
                jj = (np.arange(Kc)[:, None] * TILE + np.arange(TILE)[None, :])
                w16 = np.zeros((16, NI // 16), np.int16)
                w16[(jj % 16).ravel(), (jj // 16).ravel()] = qi[:, k0:k0 + Kc].T.ravel()
                cs = 16 * (co + k0)
                idx16[c][:, cs:cs + NI // 16] = np.tile(w16, (8, 1))
                k0 += Kc
            b0 = 9 * co
            # e4: duplicated edge features [e0,e1,e0,e1]
            ev = np.where(vm[:, :, None], e[ebs], 0.0).astype(np.float32)
            e4 = np.concatenate([ev, ev], axis=2)            # [128, K, 4]
            blob[c][:, b0:b0 + 4 * K] = e4.reshape(TILE, 4 * K)
            # qm one-hot [128, K, 4]
            qm = np.zeros((TILE, K, 4), np.float32)
            pp, kk = np.nonzero(vm)
            qm[pp, kk, qsel_all[eb[pp, kk]]] = 1.0
            blob[c][:, b0 + 4 * K:b0 + 8 * K] = qm.reshape(TILE, 4 * K)
            blob[c][:, b0 + 8 * K:b0 + 9 * K] = np.where(vm, 0.0, NEG_BIAS)

    return K_sched, col_off, idx16, blob, hsh, node_at


def _build(K_sched, col_off):
    import concourse.bass as bass
    import concourse.bacc as bacc
    import concourse.mybir as mybir
    from concourse import library_config

    DT = mybir.dt.float32
    AF = mybir.ActivationFunctionType
    OP = mybir.AluOpType
    AX = mybir.AxisListType
    Ksum = int(K_sched.sum())
    KMAX = int(K_sched.max())
    NOCC = bool(int(os.environ.get("AGAT_NOCC", "0")))

    nc = bacc.Bacc(num_swdge_queues=4, dynamic_dma_scratch_size=32768)

    hT_ext = nc.declare_dram_parameter("hT", [IN_DIM, BLOCK], DT, isOutput=False)
    wfc_ext = nc.declare_dram_parameter("wfc", [OUT_DIM, IN_DIM], DT, isOutput=False)
    aux_ext = nc.declare_dram_parameter("aux", [1, 72], DT, isOutput=False)
    auxc_ext = nc.declare_dram_parameter("auxc", [OUT_DIM, 1], DT, isOutput=False)
    idx_ext = nc.declare_dram_parameter("idx16", [128, 16 * Ksum], mybir.dt.int16, isOutput=False)
    blob_ext = nc.declare_dram_parameter("blob", [128, 9 * Ksum], DT, isOutput=False)
    out_ext = nc.declare_dram_parameter("out", [TPC, 128, OUT_DIM], DT, isOutput=True)

    tabin = nc.dram_tensor("tabin", [BLOCK // 4, ROWW], DT)
    agtab = nc.dram_tensor("agtab", [TROWS, ROWW], DT, addr_space="Shared")

    # aux row layout (row-broadcast via ones-matmul into bc_sb):
    #   [0:4]  W4 = [W00,W01,W10,W11] (W_edge rows)
    #   [4:6]  Wa_e
    #   [6:22] Wa_src
    #   [22:38] W_e2n[:,0]
    #   [38:54] W_e2n[:,1]
    C_W4, C_WAE, C_WAS, C_E2N0, C_E2N1 = 0, 4, 6, 22, 38

    from contextlib import ExitStack
    with ExitStack() as _es:
        wfc_sb = _es.enter_context(nc.sbuf_tensor([16, IN_DIM], DT))
        wfcT_sb = _es.enter_context(nc.sbuf_tensor([IN_DIM, 16], DT))
        ones_sb = _es.enter_context(nc.sbuf_tensor([1, 128], DT))
        aux_sb = _es.enter_context(nc.sbuf_tensor([1, 72], DT))
        auxc_sb = _es.enter_context(nc.sbuf_tensor([16, 1], DT))
        bc_sb = _es.enter_context(nc.sbuf_tensor([128, 72], DT))
        ident_sb = _es.enter_context(nc.sbuf_tensor([128, 128], DT))
        t_all = _es.enter_context(nc.sbuf_tensor([128, TPC], DT))
        zT0 = _es.enter_context(nc.sbuf_tensor([16, GRP * 128], DT))
        zT1 = _es.enter_context(nc.sbuf_tensor([16, GRP * 128], DT))
        zpack = _es.enter_context(nc.sbuf_tensor([128, 8 * 16], DT))
        hT_sb = _es.enter_context(nc.sbuf_tensor([IN_DIM, BLOCK], DT))
        zg0 = _es.enter_context(nc.sbuf_tensor([128, KMAX * 64], DT))
        zg1 = _es.enter_context(nc.sbuf_tensor([128, KMAX * 64], DT))
        zg2 = _es.enter_context(nc.sbuf_tensor([128, KMAX * 64], DT))
        blob0 = _es.enter_context(nc.sbuf_tensor([128, KMAX * 9], DT))
        blob1 = _es.enter_context(nc.sbuf_tensor([128, KMAX * 9], DT))
        blob2 = _es.enter_context(nc.sbuf_tensor([128, KMAX * 9], DT))
        idx0 = _es.enter_context(nc.sbuf_tensor([128, KMAX * 16], mybir.dt.int16))
        idx1 = _es.enter_context(nc.sbuf_tensor([128, KMAX * 16], mybir.dt.int16))
        idx2 = _es.enter_context(nc.sbuf_tensor([128, KMAX * 16], mybir.dt.int16))
        zcol = _es.enter_context(nc.sbuf_tensor([128, 1], DT))
        zs0 = _es.enter_context(nc.sbuf_tensor([128, KMAX * 16], DT))
        zs1 = _es.enter_context(nc.sbuf_tensor([128, KMAX * 16], DT))
        sc16 = _es.enter_context(nc.sbuf_tensor([128, KMAX * 16], DT))
        wk0 = _es.enter_context(nc.sbuf_tensor([128, 10 * KMAX], DT))
        wk1 = _es.enter_context(nc.sbuf_tensor([128, 10 * KMAX], DT))
        smal0 = _es.enter_context(nc.sbuf_tensor([128, 8], DT))
        smal1 = _es.enter_context(nc.sbuf_tensor([128, 8], DT))
        obuf = _es.enter_context(nc.sbuf_tensor([128, 32], DT))
        otile0 = _es.enter_context(nc.sbuf_tensor([128, 16], DT))
        otile1 = _es.enter_context(nc.sbuf_tensor([128, 16], DT))
        ps_z0 = _es.enter_context(nc.psum_tensor([16, GRP * 128], DT))
        ps_z1 = _es.enter_context(nc.psum_tensor([16, GRP * 128], DT))
        ps_tr = _es.enter_context(nc.psum_tensor([128, 16], DT))
        ps_a = _es.enter_context(nc.psum_tensor([128, 1], DT))
        ps_w = _es.enter_context(nc.psum_tensor([62, 16], DT))
        ps_bc = _es.enter_context(nc.psum_tensor([128, 72], DT))
        s_in = _es.enter_context(nc.semaphore("s_in"))
        s_pe = _es.enter_context(nc.semaphore("s_pe"))
        s_dv = _es.enter_context(nc.semaphore("s_dv"))
        s_ac = _es.enter_context(nc.semaphore("s_ac"))
        s_aw = _es.enter_context(nc.semaphore("s_aw"))
        s_gp = _es.enter_context(nc.semaphore("s_gp"))
        s_g = [_es.enter_context(nc.semaphore(f"s_g{q}")) for q in range(4)]
        s_cc = _es.enter_context(nc.semaphore("s_cc"))
        s_ot = [_es.enter_context(nc.semaphore(f"s_ot{p}")) for p in range(2)]
        s_init = _es.enter_context(nc.semaphore("s_init"))
        s_tl = [_es.enter_context(nc.semaphore(f"s_tl{p}")) for p in range(3)]
        block = _es.enter_context(nc.Block())

        zgs, blobs, idxs = [zg0, zg1, zg2], [blob0, blob1, blob2], [idx0, idx1, idx2]
        zss, wks, smals = [zs0, zs1], [wk0, wk1], [smal0, smal1]
        zTs, ps_zs = [zT0, zT1], [ps_z0, ps_z1]
        otiles = [otile0, otile1]

        # ---------------- stage-A sem ledger (python side) ----------------
        # s_pe increments: 1 ps_bc, 2 ps_w, then per event below.
        # s_dv increments: 1 bc copy, 2 wfcT copy, then per event below.
        pe_cnt = 2
        dv_cnt = 2
        PE_Z = [0] * NGRP       # s_pe value after ps_z matmul of group g
        PE_TR = [0] * TPC       # after transpose of tile i
        PE_A = [0] * TPC        # after ps_a matmul of tile i
        DV_ZT = [0] * NGRP      # s_dv after zT copy of group g
        DV_PK = [0] * TPC       # after zpack copy of tile i
        DV_TA = [0] * TPC       # after t_all copy of tile i
        for g in range(NGRP):
            pe_cnt += 1
            PE_Z[g] = pe_cnt
            dv_cnt += 1
            DV_ZT[g] = dv_cnt
            for i in range(g * GRP, min((g + 1) * GRP, TPC)):
                pe_cnt += 1
                PE_TR[i] = pe_cnt
                pe_cnt += 1
                PE_A[i] = pe_cnt
                dv_cnt += 1
                DV_PK[i] = dv_cnt
                dv_cnt += 1
                DV_TA[i] = dv_cnt
        DVA_END = dv_cnt

        # stage-B s_dv ladder: per tile, incs: SEL (zg free), AW implicit via
        # s_aw, OT (otile written)
        SEL = [0] * TPC
        OT = [0] * TPC
        _c = DVA_END
        for i in range(TPC):
            _c += 1
            SEL[i] = _c
            _c += 1
            OT[i] = _c

        ncalls = [(int(K) + MAXC - 1) // MAXC for K in K_sched]
        qcnt = [0, 0, 0, 0]
        qsnap = []
        call_hist = []

        def TL(i):
            return 32 * (i // 3 + 1)

        @block.sync
        def _(sy: bass.BassEngine):
            sy.dma_start(out=hT_sb[:], in_=hT_ext[:]).then_inc(s_in, 16)
            sy.dma_start(out=wfc_sb[:], in_=wfc_ext[:]).then_inc(s_in, 16)
            sy.dma_start(out=aux_sb[:], in_=aux_ext[:]).then_inc(s_in, 16)
            sy.dma_start(out=auxc_sb[:], in_=auxc_ext[:]).then_inc(s_in, 16)
            for b in range(3):
                if b < TPC:
                    ko, K = int(col_off[b]), int(K_sched[b])
                    sy.dma_start(out=idxs[b][:, :16 * K],
                                 in_=idx_ext[:, 16 * ko:16 * (ko + K)]).then_inc(s_tl[b], 16)
                    sy.dma_start(out=blobs[b][:, :9 * K],
                                 in_=blob_ext[:, 9 * ko:9 * (ko + K)]).then_inc(s_tl[b], 16)
            for i in range(3, TPC + 3):
                if i < TPC:
                    # blob/idx buffer i%3 free once tile i-3's bias add done
                    sy.wait_ge(s_aw, i - 2)
                    K = int(K_sched[i])
                    co = int(col_off[i])
                    b = i % 3
                    sy.dma_start(out=idxs[b][:, :16 * K],
                                 in_=idx_ext[:, 16 * co:16 * (co + K)]).then_inc(s_tl[b], 16)
                    sy.dma_start(out=blobs[b][:, :9 * K],
                                 in_=blob_ext[:, 9 * co:9 * (co + K)]).then_inc(s_tl[b], 16)
                if i - 3 + 2 < TPC + 2:
                    j = i - 3
                    sy.wait_ge(s_dv, OT[j])
                    sy.dma_start(out=out_ext[j],
                                 in_=otiles[j % 2][:]).then_inc(s_ot[j % 2], 16)

        @block.gpsimd
        def _(gp: bass.BassEngine):
            gp.load_library(library_config.mlp)
            gp.memset(zcol[:], 0.0).then_inc(s_init, 1)
            gp.memset(ones_sb[:], 1.0).then_inc(s_init, 1)
            gp.memset(ident_sb[:], 0.0).then_inc(s_init, 1)
            gp.wait_ge(s_init, 3)
            gp.affine_select(
                out=ident_sb[:], in_=ident_sb[:],
                compare_op=mybir.AluOpType.not_equal,
                fill=1.0, base=0, pattern=[[-1, 128]],
                channel_multiplier=1,
            ).then_inc(s_init, 1)   # s_init -> 4
            # pack-group DMAs to tabin: every 8 tiles (row col = q*16+d)
            NPG = (TPC + 7) // 8
            for j in range(NPG):
                nch = min(8, TPC - 8 * j)
                last = 8 * j + nch - 1
                gp.wait_ge(s_dv, DV_PK[last])
                gp.dma_start(
                    out=tabin[32 * 8 * j: 32 * 8 * j + 32 * nch, :].rearrange(
                        "(jj pp) (qq d) -> pp qq jj d", pp=32, qq=4),
                    in_=zpack[:, :nch * 16].rearrange("p (jj d) -> p jj d", d=16),
                ).then_inc(s_gp, 16)
            gp.wait_ge(s_gp, 16 * NPG)
            if NOCC:
                gp.dma_start(out=agtab[:BLOCK // 4, :], in_=tabin[:]).then_inc(s_cc, 16)
                gp.wait_ge(s_cc, 16)
            else:
                gp.collective_compute(
                    "AllGather", mybir.AluOpType.bypass,
                    replica_groups=[list(range(N_CORES))],
                    ins=[tabin[:]], outs=[agtab[:]],
                ).then_inc(s_cc)
                gp.wait_ge(s_cc, 1)
            call_no = 0
            for i in range(TPC):
                b = i % 3
                K = int(K_sched[i])
                gp.wait_ge(s_tl[b], TL(i))
                if i >= 3:
                    gp.wait_ge(s_dv, SEL[i - 3])
                k0 = 0
                while k0 < K:
                    Kc = min(MAXC, K - k0)
                    NI = Kc * TILE
                    q = call_no % 4
                    gp.dma_gather(
                        out_ap=zgs[b][:, 64 * k0:64 * (k0 + Kc)].rearrange(
                            "p (k w) -> p k w", w=64),
                        in_ap=agtab[:],
                        idxs_ap=idxs[b][:, 16 * k0:16 * k0 + NI // 16],
                        num_idxs=NI,
                        num_idxs_reg=NI,
                        elem_size=ROWW,
                        elem_step=ROWW,
                        queue_num=q,
                        single_packet=bool(int(os.environ.get("AGAT_SP", "1"))),
                    ).then_inc(s_g[q], 16)
                    qcnt[q] += 1
                    call_hist.append((q, qcnt[q]))
                    call_no += 1
                    if len(call_hist) > 8:
                        oq, ocnt = call_hist[-9]
                        gp.wait_ge(s_g[oq], 16 * ocnt)
                    k0 += Kc
                qsnap.append(tuple(qcnt))

        @block.tensor
        def _(te: bass.BassEngine):
            te.wait_ge(s_in, 16 * 4)
            te.wait_ge(s_init, 4)
            te.matmul(ps_bc[:], lhsT=ones_sb[:], rhs=aux_sb[:], start=True,
                      stop=True).then_inc(s_pe)                       # pe=1
            te.transpose(ps_w[:], in_=wfc_sb[:],
                         identity=ident_sb[:16, :16]).then_inc(s_pe)  # pe=2
            for g in range(NGRP):
                n = min(GRP * 128, BLOCK - g * GRP * 128)
                # ps_z[g%2] free once zT copy of group g-2 done (and wfcT at 2)
                te.wait_ge(s_dv, 2 if g < 2 else DV_ZT[g - 2])
                te.matmul(ps_zs[g % 2][:, :n], lhsT=wfcT_sb[:],
                          rhs=hT_sb[:, g * GRP * 128:g * GRP * 128 + n],
                          start=True, stop=True).then_inc(s_pe)
                for i in range(g * GRP, min((g + 1) * GRP, TPC)):
                    te.wait_ge(s_dv, DV_ZT[g])
                    if i >= 1:
                        te.wait_ge(s_dv, DV_PK[i - 1])   # ps_tr free
                    sl = zTs[g % 2][:, (i - g * GRP) * 128:(i - g * GRP) * 128 + 128]
                    te.transpose(ps_tr[:], in_=sl,
                                 identity=ident_sb[:16, :16]).then_inc(s_pe)
                    if i >= 1:
                        te.wait_ge(s_dv, DV_TA[i - 1])   # ps_a free
                    te.matmul(ps_a[:], lhsT=sl, rhs=auxc_sb[:],
                              start=True, stop=True).then_inc(s_pe)

        @block.scalar
        def _(sc: bass.BassEngine):
            for i in range(TPC):
                b = i % 2
                K = int(K_sched[i])
                a_v = wks[b][:, 0:K]
                w_v = wks[b][:, 2 * KMAX:2 * KMAX + K]
                sc.wait_ge(s_aw, i + 1)
                sc.activation(w_v, a_v, AF.Exp,
                              accum_out=smals[b][:, 0:1]).then_inc(s_ac)

        @block.vector
        def _(ve: bass.BassEngine):
            ve.wait_ge(s_pe, 1)
            ve.tensor_copy(bc_sb[:], ps_bc[:]).then_inc(s_dv)          # dv=1
            ve.wait_ge(s_pe, 2)
            ve.tensor_copy(wfcT_sb[:], ps_w[:]).then_inc(s_dv)         # dv=2
            # -------- stage A --------
            for g in range(NGRP):
                n = min(GRP * 128, BLOCK - g * GRP * 128)
                ve.wait_ge(s_pe, PE_Z[g])
                ve.tensor_copy(zTs[g % 2][:, :n], ps_zs[g % 2][:, :n]).then_inc(s_dv)
                for i in range(g * GRP, min((g + 1) * GRP, TPC)):
                    ve.wait_ge(s_pe, PE_TR[i])
                    if i % 8 == 0 and i > 0:
                        ve.wait_ge(s_gp, 16 * (i // 8))  # zpack group flushed
                    ve.tensor_copy(zpack[:, 16 * (i % 8):16 * (i % 8) + 16],
                                   ps_tr[:]).then_inc(s_dv)
                    ve.wait_ge(s_pe, PE_A[i])
                    ve.tensor_copy(t_all[:, i:i + 1], ps_a[:]).then_inc(s_dv)
            # -------- stage B --------
            for i in range(TPC):
                b = i % 2
                b3 = i % 3
                K = int(K_sched[i])
                zg, bl, zs = zgs[b3], blobs[b3], zss[b]
                e4_v = bl[:, 0:4 * K]
                qm_v = bl[:, 4 * K:8 * K]
                bias_v = bl[:, 8 * K:9 * K]
                a_v = wks[b][:, 0:K]
                w_v = wks[b][:, 2 * KMAX:2 * KMAX + K]
                exv = wks[b][:, 3 * KMAX:3 * KMAX + 4 * K]       # [K,4] scratch
                ex_v = wks[b][:, 8 * KMAX:8 * KMAX + 2 * K]      # [K,2]
                sv_v = wks[b][:, KMAX:KMAX + K]                  # dot result
                den, rden, wex = smals[b][:, 0:1], smals[b][:, 1:2], smals[b][:, 2:4]

                # zg views: row col = q*16+d
                zg_flat = zg[:, :64 * K].rearrange("p (kq d) -> p kq d", d=16)
                zg_half = zg[:, :64 * K].rearrange("p (k two qd) -> p k two qd",
                                                   two=2, qd=32)
                zg_kqd = zg[:, :64 * K].rearrange("p (k q d) -> p k q d", q=4, d=16)
                zs3 = zs[:, :16 * K].rearrange("p (k d) -> p k d", d=16)
                sc3 = sc16[:, :16 * K].rearrange("p (k d) -> p k d", d=16)

                for q in range(4):
                    if qsnap[i][q] > 0:
                        ve.wait_ge(s_g[q], 16 * qsnap[i][q])
                ve.wait_ge(s_tl[b3], TL(i))
                ve.drain()
                # L0: select-mult (in place) + ex mult (independent)
                ve.tensor_tensor(
                    out=zg_flat, in0=zg_flat,
                    in1=qm_v.to_broadcast([128, 4 * K, 16]),
                    op=OP.mult)
                ve.tensor_tensor(
                    out=exv.rearrange("p (k x) -> p k x", x=4),
                    in0=e4_v.rearrange("p (k x) -> p k x", x=4),
                    in1=bc_sb[:, C_W4:C_W4 + 4].to_broadcast([128, 4, K])
                        .rearrange("p x k -> p k x"),
                    op=OP.mult)
                ve.drain()
                # L1: add tree level 1 (q0,q1 += q2,q3) + ex pair reduce
                ve.tensor_tensor(
                    out=zg_half[:, :, 0, :], in0=zg_half[:, :, 0, :],
                    in1=zg_half[:, :, 1, :], op=OP.add)
                ve.tensor_reduce(
                    out=ex_v.rearrange("p (k x) -> p k x", x=2),
                    in_=exv.rearrange("p (k x two) -> p k x two", x=2, two=2),
                    axis=AX.X, op=OP.add)
                ve.drain()
                # L2: add tree level 2 -> zs3 ; ae mult
                ve.tensor_tensor(
                    out=zs3, in0=zg_kqd[:, :, 0, :], in1=zg_kqd[:, :, 1, :],
                    op=OP.add).then_inc(s_dv)                    # SEL[i]: zg free
                ve.tensor_tensor(
                    out=exv[:, 0:2 * K].rearrange("p (k x) -> p k x", x=2),
                    in0=ex_v.rearrange("p (k x) -> p k x", x=2),
                    in1=bc_sb[:, C_WAE:C_WAE + 2].to_broadcast([128, 2, K])
                        .rearrange("p x k -> p k x"),
                    op=OP.mult)
                ve.drain()
                # L3: dot mult + ae reduce
                ve.tensor_tensor(
                    out=sc3, in0=zs3,
                    in1=bc_sb[:, C_WAS:C_WAS + 16].to_broadcast([128, 16, K])
                        .rearrange("p d k -> p k d"),
                    op=OP.mult)
                ve.tensor_reduce(
                    out=exv[:, 2 * K:3 * K],
                    in_=exv[:, 0:2 * K].rearrange("p (k x) -> p k x", x=2),
                    axis=AX.X, op=OP.add)
                ve.drain()
                # L4: dot reduce
                ve.tensor_reduce(out=sv_v, in_=sc3, axis=AX.X, op=OP.add)
                ve.drain()
                # L5: a = (sv + t_dst) + ae ; then + bias
                ve.scalar_tensor_tensor(
                    out=a_v, in0=sv_v, scalar=t_all[:, i:i + 1],
                    in1=exv[:, 2 * K:3 * K], op0=OP.add, op1=OP.add)
                ve.drain()
                ve.tensor_tensor(out=a_v, in0=a_v, in1=bias_v, op=OP.add)
                ve.drain()
                ve.scalar_tensor_tensor(
                    out=sv_v, in0=a_v, scalar=0.01,
                    in1=zcol[:].to_broadcast([128, K]),
                    op0=OP.mult, op1=OP.add)
                ve.drain()
                ve.tensor_tensor(out=a_v, in0=a_v, in1=sv_v,
                                 op=OP.max).then_inc(s_aw)
                # scalar engine computes w=exp(a), den
                ve.wait_ge(s_ac, i + 1)
                # L6: weighted sums
                ve.tensor_tensor(out=sc3, in0=zs3,
                                 in1=w_v.to_broadcast([128, K, 16]), op=OP.mult)
                ve.tensor_tensor(
                    out=exv[:, 0:2 * K].rearrange("p (k x) -> p k x", x=2),
                    in0=ex_v.rearrange("p (k x) -> p k x", x=2),
                    in1=w_v.to_broadcast([128, K, 2]), op=OP.mult)
                ve.scalar_tensor_tensor(
                    out=den, in0=den, scalar=1e-30,
                    in1=zcol[:, 0:1], op0=OP.add, op1=OP.add)
                ve.drain()
                ve.tensor_reduce(
                    out=obuf[:, 0:16],
                    in_=sc16[:, :16 * K].rearrange("p (k d) -> p d k", d=16),
                    axis=AX.X, op=OP.add)
                ve.tensor_reduce(
                    out=wex,
                    in_=exv[:, 0:2 * K].rearrange("p (k x) -> p x k", x=2),
                    axis=AX.X, op=OP.add)
                ve.reciprocal(rden, den)
                ve.drain()
                # L7: obuf = e2n0*wex0 + acc ; then e2n1*wex1 + ...
                ve.scalar_tensor_tensor(
                    out=obuf[:, 16:32], in0=bc_sb[:, C_E2N0:C_E2N0 + 16],
                    scalar=wex[:, 0:1], in1=obuf[:, 0:16],
                    op0=OP.mult, op1=OP.add)
                ve.drain()
                ve.scalar_tensor_tensor(
                    out=obuf[:, 0:16], in0=bc_sb[:, C_E2N1:C_E2N1 + 16],
                    scalar=wex[:, 1:2], in1=obuf[:, 16:32],
                    op0=OP.mult, op1=OP.add)
                if i >= 2:
                    ve.wait_ge(s_ot[b], 16 * (i // 2))
                ve.drain()
                ve.scalar_tensor_tensor(
                    out=otiles[b][:], in0=obuf[:, 0:16], scalar=rden,
                    in1=zcol[:].to_broadcast([128, 16]),
                    op0=OP.mult, op1=OP.add).then_inc(s_dv)      # OT[i]

    nc.compile()
    return nc


_CACHE = {}


def kernel(h, e, src, dst, W_fc, W_attn, W_edge, W_e2n):
    import concourse.bass_utils as bu

    h = np.asarray(h, np.float32)
    e = np.asarray(e, np.float32)
    src = np.asarray(src, np.int64)
    dst = np.asarray(dst, np.int64)
    W_fc = np.asarray(W_fc, np.float32)
    W_attn = np.asarray(W_attn, np.float32)
    W_edge = np.asarray(W_edge, np.float32)
    W_e2n = np.asarray(W_e2n, np.float32)

    K_sched, col_off, idx16, blob, hsh, node_at = _host_prep(h, e, src, dst)

    key = tuple(K_sched.tolist())
    if key not in _CACHE:
        _CACHE[key] = _build(K_sched, col_off)
    nc = _CACHE[key]

    aux = np.zeros((1, 72), np.float32)
    aux[0, 0:2] = W_edge[0, :]
    aux[0, 2:4] = W_edge[1, :]
    aux[0, 4:6] = W_attn[0, 2 * OUT_DIM:]
    aux[0, 6:22] = W_attn[0, :OUT_DIM]
    aux[0, 22:38] = W_e2n[:, 0]
    aux[0, 38:54] = W_e2n[:, 1]
    auxc = W_attn[0, OUT_DIM:2 * OUT_DIM].reshape(OUT_DIM, 1).astype(np.float32)

    in_maps = [{
        "hT": hsh[c], "wfc": W_fc, "aux": aux, "auxc": auxc,
        "idx16": idx16[c], "blob": blob[c],
    } for c in range(N_CORES)]
    trace = bool(int(os.environ.get("AGAT_TRACE", "0")))
    if trace:
        _install_ntff_shim()
    res = bu.run_bass_kernel_spmd(nc, in_maps, core_ids=list(range(N_CORES)),
                                  trace=trace)
    global LAST_EXEC_NS
    LAST_EXEC_NS = res.exec_time_ns

    out = np.zeros((N_NODES, OUT_DIM), np.float32)
    for c in range(N_CORES):
        oc = res.results[c]["out"].reshape(BLOCK, OUT_DIM)
        rows = node_at[c]
        v = rows >= 0
        out[rows[v]] = oc[v]
    return out


# revision 10
# speedup vs baseline: 1.0089x; 1.0089x over previous
"""AGAT layer (GNN message passing) on 8 TRN2 NeuronCores.

Strategy v2 (dst-sharded, padded-CSR, single collective, lean DVE):
  - Nodes degree-sorted into 128-node tiles; tile groups of 8 dealt
    round-robin to cores (identical SPMD schedule, balanced edges).
  - Each core computes z = h @ W_fc.T for its 12544-node block on PE,
    packs 4 nodes per 256B row with (d,q)-INTERLEAVED layout
    (row col = d*4+q), AllGathers the table.
  - Per tile, 256B rows are fetched with dma_gather (int16 row idx);
    the 1-of-4 node select = qm mask-mult + 2-level contiguous add tree.
  - Attention: a_src dot on DVE, a_dst via per-tile PE matmul,
    leaky-relu + exp + denominator accumulation on the Scalar engine.
  - dst nodes on partitions, edge slots on the free dim; each core owns
    its dst nodes -> no accumulator all-reduce.
"""
import os
import numpy as np

LAST_EXEC_NS = None


def _install_ntff_shim():
    """Register the NTFF profile hook bass_utils expects under axon."""
    import sys
    import types
    import antenv
    if "antenv.axon_hooks" in sys.modules:
        return
    mod = types.ModuleType("antenv.axon_hooks")
    mod._hook = None
    mod.set_axon_ntff_profile_hook = lambda h: setattr(mod, "_hook", h)
    mod.get_axon_ntff_profile_hook = lambda: mod._hook
    sys.modules["antenv.axon_hooks"] = mod
    antenv.axon_hooks = mod
    try:
        from trn_agent_boot.trn_boot import _ntff_profile_via_ctypes
        mod.set_axon_ntff_profile_hook(
            _ntff_profile_via_ctypes("/opt/axon/libaxon_pjrt.so"))
    except Exception:
        pass


N_NODES = 100000
N_EDGES = 3200000
IN_DIM = 62
OUT_DIM = 16
EDGE_DIM = 2
N_CORES = 8
TILE = 128
TPC = 98                      # tiles per core
NT = N_CORES * TPC            # 784 tiles
NPAD = NT * TILE              # 100352 padded nodes
BLOCK = TPC * TILE            # 12544 nodes per core block
TROWS = NPAD // 4             # 25088 packed table rows
ROWW = 64                     # table row width in f32 (4 nodes x 16)
MAXC = int(os.environ.get("AGAT_MAXC", "8"))  # max k-cols per gather call
GWIN = max(2, 8192 // (MAXC * 128))           # outstanding gather calls
NEG_BIAS = -6000.0
GRP = 4                       # tiles per z-matmul group (free dim 512)
NGRP = (TPC + GRP - 1) // GRP


def _host_prep(h, e, src, dst):
    deg = np.bincount(dst, minlength=N_NODES)
    order = np.argsort(-deg, kind="stable").astype(np.int64)
    order_pad = np.concatenate([order, np.full(NPAD - N_NODES, -1, np.int64)])

    degp = np.concatenate([deg, np.zeros(NPAD - N_NODES, np.int64)])
    deg_of = np.where(order_pad >= 0, degp[np.maximum(order_pad, 0)], 0)
    Kg = deg_of.reshape(NT, TILE).max(axis=1)
    K_sched = np.maximum(Kg.reshape(TPC, N_CORES).max(axis=1), 1).astype(np.int64)

    # table layout: core c block rows [c*BLOCK, (c+1)*BLOCK); tile i of core c
    # = global tile 8i+c at rows c*BLOCK + i*TILE + p
    node_at = np.full((N_CORES, BLOCK), -1, np.int64)
    tabpos = np.full(N_NODES, -1, np.int64)
    for g in range(NT):
        i, c = divmod(g, N_CORES)
        nodes = order_pad[g * TILE:(g + 1) * TILE]
        node_at[c, i * TILE:(i + 1) * TILE] = nodes
        valid = nodes >= 0
        tp = c * BLOCK + i * TILE + np.nonzero(valid)[0]
        tabpos[nodes[valid]] = tp

    e_order = np.argsort(dst, kind="stable")
    csr_off = np.zeros(N_NODES + 1, np.int64)
    np.cumsum(deg, out=csr_off[1:])

    Ksum = int(K_sched.sum())
    col_off = np.zeros(TPC + 1, np.int64)
    np.cumsum(K_sched, out=col_off[1:])

    idx16 = np.zeros((N_CORES, 128, 16 * Ksum), np.int16)
    blob = np.zeros((N_CORES, 128, 9 * Ksum), np.float32)
    hsh = np.zeros((N_CORES, IN_DIM, BLOCK), np.float32)

    src_pos = tabpos[src]
    qrow_all = (src_pos // 4).astype(np.int32)
    qsel_all = (src_pos % 4).astype(np.int64)

    for c in range(N_CORES):
        hrows = node_at[c]
        hv = hrows >= 0
        hsh[c][:, hv] = h[hrows[hv]].T
        for i in range(TPC):
            K = int(K_sched[i])
            nodes = node_at[c, i * TILE:(i + 1) * TILE]
            co = int(col_off[i])
            eb = np.full((TILE, K), -1, np.int64)
            for p in range(TILE):
                n = nodes[p]
                if n < 0:
                    continue
                eds = e_order[csr_off[n]:csr_off[n + 1]]
                eb[p, :len(eds)] = eds
            vm = eb >= 0
            ebs = np.maximum(eb, 0)
            qi = np.where(vm, qrow_all[ebs], 0).astype(np.int16)  # [128, K]
            k0 = 0
            while k0 < K:
                Kc = min(MAXC, K - k0)
                NI = Kc * TILE
                jj = (np.arange(Kc)[:, None] * TILE + np.arange(TILE)[None, :])
                w16 = np.zeros((16, NI // 16), np.int16)
                w16[(jj % 16).ravel(), (jj // 16).ravel()] = qi[:, k0:k0 + Kc].T.ravel()
                cs = 16 * (co + k0)
                idx16[c][:, cs:cs + NI // 16] = np.tile(w16, (8, 1))
                k0 += Kc
            b0 = 9 * co
            # e4: duplicated edge features [e0,e1,e0,e1]
            ev = np.where(vm[:, :, None], e[ebs], 0.0).astype(np.float32)
            e4 = np.concatenate([ev, ev], axis=2)            # [128, K, 4]
            blob[c][:, b0:b0 + 4 * K] = e4.reshape(TILE, 4 * K)
            # qm one-hot [128, K, 4]
            qm = np.zeros((TILE, K, 4), np.float32)
            pp, kk = np.nonzero(vm)
            qm[pp, kk, qsel_all[eb[pp, kk]]] = 1.0
            blob[c][:, b0 + 4 * K:b0 + 8 * K] = qm.reshape(TILE, 4 * K)
            blob[c][:, b0 + 8 * K:b0 + 9 * K] = np.where(vm, 0.0, NEG_BIAS)

    return K_sched, col_off, idx16, blob, hsh, node_at


def _build(K_sched, col_off):
    import concourse.bass as bass
    import concourse.bacc as bacc
    import concourse.mybir as mybir
    from concourse import library_config

    DT = mybir.dt.float32
    AF = mybir.ActivationFunctionType
    OP = mybir.AluOpType
    AX = mybir.AxisListType
    Ksum = int(K_sched.sum())
    KMAX = int(K_sched.max())
    NOCC = bool(int(os.environ.get("AGAT_NOCC", "0")))

    nc = bacc.Bacc(num_swdge_queues=4,
                   dynamic_dma_scratch_size=int(os.environ.get("AGAT_SCR", "32768")))

    hT_ext = nc.declare_dram_parameter("hT", [IN_DIM, BLOCK], DT, isOutput=False)
    wfc_ext = nc.declare_dram_parameter("wfc", [OUT_DIM, IN_DIM], DT, isOutput=False)
    aux_ext = nc.declare_dram_parameter("aux", [1, 72], DT, isOutput=False)
    auxc_ext = nc.declare_dram_parameter("auxc", [OUT_DIM, 1], DT, isOutput=False)
    idx_ext = nc.declare_dram_parameter("idx16", [128, 16 * Ksum], mybir.dt.int16, isOutput=False)
    blob_ext = nc.declare_dram_parameter("blob", [128, 9 * Ksum], DT, isOutput=False)
    out_ext = nc.declare_dram_parameter("out", [TPC, 128, OUT_DIM], DT, isOutput=True)

    tabin = nc.dram_tensor("tabin", [BLOCK // 4, ROWW], DT)
    agtab = nc.dram_tensor("agtab", [TROWS, ROWW], DT, addr_space="Shared")

    # aux row layout (row-broadcast via ones-matmul into bc_sb):
    #   [0:4]  W4 = [W00,W01,W10,W11] (W_edge rows)
    #   [4:6]  Wa_e
    #   [6:22] Wa_src
    #   [22:38] W_e2n[:,0]
    #   [38:54] W_e2n[:,1]
    C_W4, C_WAE, C_WAS, C_E2N0, C_E2N1 = 0, 4, 6, 22, 38

    from contextlib import ExitStack
    with ExitStack() as _es:
        wfc_sb = _es.enter_context(nc.sbuf_tensor([16, IN_DIM], DT))
        wfcT_sb = _es.enter_context(nc.sbuf_tensor([IN_DIM, 16], DT))
        ones_sb = _es.enter_context(nc.sbuf_tensor([1, 128], DT))
        aux_sb = _es.enter_context(nc.sbuf_tensor([1, 72], DT))
        auxc_sb = _es.enter_context(nc.sbuf_tensor([16, 1], DT))
        bc_sb = _es.enter_context(nc.sbuf_tensor([128, 72], DT))
        ident_sb = _es.enter_context(nc.sbuf_tensor([128, 128], DT))
        t_all = _es.enter_context(nc.sbuf_tensor([128, TPC], DT))
        zT0 = _es.enter_context(nc.sbuf_tensor([16, GRP * 128], DT))
        zT1 = _es.enter_context(nc.sbuf_tensor([16, GRP * 128], DT))
        zpack = _es.enter_context(nc.sbuf_tensor([128, 8 * 16], DT))
        hT_sb = _es.enter_context(nc.sbuf_tensor([IN_DIM, BLOCK], DT))
        zg0 = _es.enter_context(nc.sbuf_tensor([128, KMAX * 64], DT))
        zg1 = _es.enter_context(nc.sbuf_tensor([128, KMAX * 64], DT))
        zg2 = _es.enter_context(nc.sbuf_tensor([128, KMAX * 64], DT))
        blob0 = _es.enter_context(nc.sbuf_tensor([128, KMAX * 9], DT))
        blob1 = _es.enter_context(nc.sbuf_tensor([128, KMAX * 9], DT))
        blob2 = _es.enter_context(nc.sbuf_tensor([128, KMAX * 9], DT))
        idx0 = _es.enter_context(nc.sbuf_tensor([128, KMAX * 16], mybir.dt.int16))
        idx1 = _es.enter_context(nc.sbuf_tensor([128, KMAX * 16], mybir.dt.int16))
        idx2 = _es.enter_context(nc.sbuf_tensor([128, KMAX * 16], mybir.dt.int16))
        zcol = _es.enter_context(nc.sbuf_tensor([128, 1], DT))
        zs0 = _es.enter_context(nc.sbuf_tensor([128, KMAX * 16], DT))
        zs1 = _es.enter_context(nc.sbuf_tensor([128, KMAX * 16], DT))
        sc16 = _es.enter_context(nc.sbuf_tensor([128, KMAX * 16], DT))
        wk0 = _es.enter_context(nc.sbuf_tensor([128, 10 * KMAX], DT))
        wk1 = _es.enter_context(nc.sbuf_tensor([128, 10 * KMAX], DT))
        smal0 = _es.enter_context(nc.sbuf_tensor([128, 8], DT))
        smal1 = _es.enter_context(nc.sbuf_tensor([128, 8], DT))
        obuf = _es.enter_context(nc.sbuf_tensor([128, 32], DT))
        otile0 = _es.enter_context(nc.sbuf_tensor([128, 16], DT))
        otile1 = _es.enter_context(nc.sbuf_tensor([128, 16], DT))
        ps_z0 = _es.enter_context(nc.psum_tensor([16, GRP * 128], DT))
        ps_z1 = _es.enter_context(nc.psum_tensor([16, GRP * 128], DT))
        ps_tr = _es.enter_context(nc.psum_tensor([128, 16], DT))
        ps_a = _es.enter_context(nc.psum_tensor([128, 1], DT))
        ps_w = _es.enter_context(nc.psum_tensor([62, 16], DT))
        ps_bc = _es.enter_context(nc.psum_tensor([128, 72], DT))
        s_in = _es.enter_context(nc.semaphore("s_in"))
        s_pe = _es.enter_context(nc.semaphore("s_pe"))
        s_dv = _es.enter_context(nc.semaphore("s_dv"))
        s_ac = _es.enter_context(nc.semaphore("s_ac"))
        s_aw = _es.enter_context(nc.semaphore("s_aw"))
        s_gp = _es.enter_context(nc.semaphore("s_gp"))
        s_g = [_es.enter_context(nc.semaphore(f"s_g{q}")) for q in range(4)]
        s_cc = _es.enter_context(nc.semaphore("s_cc"))
        s_ot = [_es.enter_context(nc.semaphore(f"s_ot{p}")) for p in range(2)]
        s_init = _es.enter_context(nc.semaphore("s_init"))
        s_tl = [_es.enter_context(nc.semaphore(f"s_tl{p}")) for p in range(3)]
        block = _es.enter_context(nc.Block())

        zgs, blobs, idxs = [zg0, zg1, zg2], [blob0, blob1, blob2], [idx0, idx1, idx2]
        zss, wks, smals = [zs0, zs1], [wk0, wk1], [smal0, smal1]
        zTs, ps_zs = [zT0, zT1], [ps_z0, ps_z1]
        otiles = [otile0, otile1]

        # ---------------- stage-A sem ledger (python side) ----------------
        # s_pe increments: 1 ps_bc, 2 ps_w, then per event below.
        # s_dv increments: 1 bc copy, 2 wfcT copy, then per event below.
        pe_cnt = 2
        dv_cnt = 2
        PE_Z = [0] * NGRP       # s_pe value after ps_z matmul of group g
        PE_TR = [0] * TPC       # after transpose of tile i
        PE_A = [0] * TPC        # after ps_a matmul of tile i
        DV_ZT = [0] * NGRP      # s_dv after zT copy of group g
        DV_PK = [0] * TPC       # after zpack copy of tile i
        DV_TA = [0] * TPC       # after t_all copy of tile i
        for g in range(NGRP):
            pe_cnt += 1
            PE_Z[g] = pe_cnt
            dv_cnt += 1
            DV_ZT[g] = dv_cnt
            for i in range(g * GRP, min((g + 1) * GRP, TPC)):
                pe_cnt += 1
                PE_TR[i] = pe_cnt
                pe_cnt += 1
                PE_A[i] = pe_cnt
                dv_cnt += 1
                DV_PK[i] = dv_cnt
                dv_cnt += 1
                DV_TA[i] = dv_cnt
        DVA_END = dv_cnt

        # stage-B s_dv ladder: per tile, incs: SEL (zg free), AW implicit via
        # s_aw, OT (otile written)
        SEL = [0] * TPC
        OT = [0] * TPC
        _c = DVA_END
        for i in range(TPC):
            _c += 1
            SEL[i] = _c
            _c += 1
            OT[i] = _c

        ncalls = [(int(K) + MAXC - 1) // MAXC for K in K_sched]
        qcnt = [0, 0, 0, 0]
        qsnap = []
        call_hist = []

        def TL(i):
            return 32 * (i // 3 + 1)

        @block.sync
        def _(sy: bass.BassEngine):
            sy.dma_start(out=hT_sb[:], in_=hT_ext[:]).then_inc(s_in, 16)
            sy.dma_start(out=wfc_sb[:], in_=wfc_ext[:]).then_inc(s_in, 16)
            sy.dma_start(out=aux_sb[:], in_=aux_ext[:]).then_inc(s_in, 16)
            sy.dma_start(out=auxc_sb[:], in_=auxc_ext[:]).then_inc(s_in, 16)
            for b in range(3):
                if b < TPC:
                    ko, K = int(col_off[b]), int(K_sched[b])
                    sy.dma_start(out=idxs[b][:, :16 * K],
                                 in_=idx_ext[:, 16 * ko:16 * (ko + K)]).then_inc(s_tl[b], 16)
                    sy.dma_start(out=blobs[b][:, :9 * K],
                                 in_=blob_ext[:, 9 * ko:9 * (ko + K)]).then_inc(s_tl[b], 16)
            for i in range(3, TPC + 3):
                if i < TPC:
                    # blob/idx buffer i%3 free once tile i-3's bias add done
                    sy.wait_ge(s_aw, i - 2)
                    K = int(K_sched[i])
                    co = int(col_off[i])
                    b = i % 3
                    sy.dma_start(out=idxs[b][:, :16 * K],
                                 in_=idx_ext[:, 16 * co:16 * (co + K)]).then_inc(s_tl[b], 16)
                    sy.dma_start(out=blobs[b][:, :9 * K],
                                 in_=blob_ext[:, 9 * co:9 * (co + K)]).then_inc(s_tl[b], 16)
                if i - 3 + 2 < TPC + 2:
                    j = i - 3
                    sy.wait_ge(s_dv, OT[j])
                    sy.dma_start(out=out_ext[j],
                                 in_=otiles[j % 2][:]).then_inc(s_ot[j % 2], 16)

        @block.gpsimd
        def _(gp: bass.BassEngine):
            gp.load_library(library_config.mlp)
            gp.memset(zcol[:], 0.0).then_inc(s_init, 1)
            gp.memset(ones_sb[:], 1.0).then_inc(s_init, 1)
            gp.memset(ident_sb[:], 0.0).then_inc(s_init, 1)
            gp.wait_ge(s_init, 3)
            gp.affine_select(
                out=ident_sb[:], in_=ident_sb[:],
                compare_op=mybir.AluOpType.not_equal,
                fill=1.0, base=0, pattern=[[-1, 128]],
                channel_multiplier=1,
            ).then_inc(s_init, 1)   # s_init -> 4
            # pack-group DMAs to tabin: every 8 tiles (row col = q*16+d)
            NPG = (TPC + 7) // 8
            for j in range(NPG):
                nch = min(8, TPC - 8 * j)
                last = 8 * j + nch - 1
                gp.wait_ge(s_dv, DV_PK[last])
                gp.dma_start(
                    out=tabin[32 * 8 * j: 32 * 8 * j + 32 * nch, :].rearrange(
                        "(jj pp) (qq d) -> pp qq jj d", pp=32, qq=4),
                    in_=zpack[:, :nch * 16].rearrange("p (jj d) -> p jj d", d=16),
                ).then_inc(s_gp, 16)
            gp.wait_ge(s_gp, 16 * NPG)
            if NOCC:
                gp.dma_start(out=agtab[:BLOCK // 4, :], in_=tabin[:]).then_inc(s_cc, 16)
                gp.wait_ge(s_cc, 16)
            else:
                gp.collective_compute(
                    "AllGather", mybir.AluOpType.bypass,
                    replica_groups=[list(range(N_CORES))],
                    ins=[tabin[:]], outs=[agtab[:]],
                ).then_inc(s_cc)
                gp.wait_ge(s_cc, 1)
            call_no = 0
            for i in range(TPC):
                b = i % 3
                K = int(K_sched[i])
                gp.wait_ge(s_tl[b], TL(i))
                if i >= 3:
                    gp.wait_ge(s_dv, SEL[i - 3])
                k0 = 0
                while k0 < K:
                    Kc = min(MAXC, K - k0)
                    NI = Kc * TILE
                    q = call_no % 4
                    gp.dma_gather(
                        out_ap=zgs[b][:, 64 * k0:64 * (k0 + Kc)].rearrange(
                            "p (k w) -> p k w", w=64),
                        in_ap=agtab[:],
                        idxs_ap=idxs[b][:, 16 * k0:16 * k0 + NI // 16],
                        num_idxs=NI,
                        num_idxs_reg=NI,
                        elem_size=ROWW,
                        elem_step=ROWW,
                        queue_num=q,
                        single_packet=bool(int(os.environ.get("AGAT_SP", "1"))),
                    ).then_inc(s_g[q], 16)
                    qcnt[q] += 1
                    call_hist.append((q, qcnt[q]))
                    call_no += 1
                    if len(call_hist) > GWIN:
                        oq, ocnt = call_hist[-(GWIN + 1)]
                        gp.wait_ge(s_g[oq], 16 * ocnt)
                    k0 += Kc
                qsnap.append(tuple(qcnt))

        @block.tensor
        def _(te: bass.BassEngine):
            te.wait_ge(s_in, 16 * 4)
            te.wait_ge(s_init, 4)
            te.matmul(ps_bc[:], lhsT=ones_sb[:], rhs=aux_sb[:], start=True,
                      stop=True).then_inc(s_pe)                       # pe=1
            te.transpose(ps_w[:], in_=wfc_sb[:],
                         identity=ident_sb[:16, :16]).then_inc(s_pe)  # pe=2
            for g in range(NGRP):
                n = min(GRP * 128, BLOCK - g * GRP * 128)
                # ps_z[g%2] free once zT copy of group g-2 done (and wfcT at 2)
                te.wait_ge(s_dv, 2 if g < 2 else DV_ZT[g - 2])
                te.matmul(ps_zs[g % 2][:, :n], lhsT=wfcT_sb[:],
                          rhs=hT_sb[:, g * GRP * 128:g * GRP * 128 + n],
                          start=True, stop=True).then_inc(s_pe)
                for i in range(g * GRP, min((g + 1) * GRP, TPC)):
                    te.wait_ge(s_dv, DV_ZT[g])
                    if i >= 1:
                        te.wait_ge(s_dv, DV_PK[i - 1])   # ps_tr free
                    sl = zTs[g % 2][:, (i - g * GRP) * 128:(i - g * GRP) * 128 + 128]
                    te.transpose(ps_tr[:], in_=sl,
                                 identity=ident_sb[:16, :16]).then_inc(s_pe)
                    if i >= 1:
                        te.wait_ge(s_dv, DV_TA[i - 1])   # ps_a free
                    te.matmul(ps_a[:], lhsT=sl, rhs=auxc_sb[:],
                              start=True, stop=True).then_inc(s_pe)

        @block.scalar
        def _(sc: bass.BassEngine):
            for i in range(TPC):
                b = i % 2
                K = int(K_sched[i])
                a_v = wks[b][:, 0:K]
                w_v = wks[b][:, 2 * KMAX:2 * KMAX + K]
                sc.wait_ge(s_aw, i + 1)
                sc.activation(w_v, a_v, AF.Exp,
                              accum_out=smals[b][:, 0:1]).then_inc(s_ac)

        @block.vector
        def _(ve: bass.BassEngine):
            ve.wait_ge(s_pe, 1)
            ve.tensor_copy(bc_sb[:], ps_bc[:]).then_inc(s_dv)          # dv=1
            ve.wait_ge(s_pe, 2)
            ve.tensor_copy(wfcT_sb[:], ps_w[:]).then_inc(s_dv)         # dv=2
            # -------- stage A --------
            for g in range(NGRP):
                n = min(GRP * 128, BLOCK - g * GRP * 128)
                ve.wait_ge(s_pe, PE_Z[g])
                ve.tensor_copy(zTs[g % 2][:, :n], ps_zs[g % 2][:, :n]).then_inc(s_dv)
                for i in range(g * GRP, min((g + 1) * GRP, TPC)):
                    ve.wait_ge(s_pe, PE_TR[i])
                    if i % 8 == 0 and i > 0:
                        ve.wait_ge(s_gp, 16 * (i // 8))  # zpack group flushed
                    ve.tensor_copy(zpack[:, 16 * (i % 8):16 * (i % 8) + 16],
                                   ps_tr[:]).then_inc(s_dv)
                    ve.wait_ge(s_pe, PE_A[i])
                    ve.tensor_copy(t_all[:, i:i + 1], ps_a[:]).then_inc(s_dv)
            # -------- stage B --------
            for i in range(TPC):
                b = i % 2
                b3 = i % 3
                K = int(K_sched[i])
                zg, bl, zs = zgs[b3], blobs[b3], zss[b]
                e4_v = bl[:, 0:4 * K]
                qm_v = bl[:, 4 * K:8 * K]
                bias_v = bl[:, 8 * K:9 * K]
                a_v = wks[b][:, 0:K]
                w_v = wks[b][:, 2 * KMAX:2 * KMAX + K]
                exv = wks[b][:, 3 * KMAX:3 * KMAX + 4 * K]       # [K,4] scratch
                ex_v = wks[b][:, 8 * KMAX:8 * KMAX + 2 * K]      # [K,2]
                sv_v = wks[b][:, KMAX:KMAX + K]                  # dot result
                den, rden, wex = smals[b][:, 0:1], smals[b][:, 1:2], smals[b][:, 2:4]

                # zg views: row col = q*16+d
                zg_flat = zg[:, :64 * K].rearrange("p (kq d) -> p kq d", d=16)
                zg_half = zg[:, :64 * K].rearrange("p (k two qd) -> p k two qd",
                                                   two=2, qd=32)
                zg_kqd = zg[:, :64 * K].rearrange("p (k q d) -> p k q d", q=4, d=16)
                zs3 = zs[:, :16 * K].rearrange("p (k d) -> p k d", d=16)
                sc3 = sc16[:, :16 * K].rearrange("p (k d) -> p k d", d=16)

                for q in range(4):
                    if qsnap[i][q] > 0:
                        ve.wait_ge(s_g[q], 16 * qsnap[i][q])
                ve.wait_ge(s_tl[b3], TL(i))
                ve.drain()
                # L0: select-mult (in place) + ex mult (independent)
                ve.tensor_tensor(
                    out=zg_flat, in0=zg_flat,
                    in1=qm_v.to_broadcast([128, 4 * K, 16]),
                    op=OP.mult)
                ve.tensor_tensor(
                    out=exv.rearrange("p (k x) -> p k x", x=4),
                    in0=e4_v.rearrange("p (k x) -> p k x", x=4),
                    in1=bc_sb[:, C_W4:C_W4 + 4].to_broadcast([128, 4, K])
                        .rearrange("p x k -> p k x"),
                    op=OP.mult)
                ve.drain()
                # L1: add tree level 1 (q0,q1 += q2,q3) + ex pair reduce
                ve.tensor_tensor(
                    out=zg_half[:, :, 0, :], in0=zg_half[:, :, 0, :],
                    in1=zg_half[:, :, 1, :], op=OP.add)
                ve.tensor_reduce(
                    out=ex_v.rearrange("p (k x) -> p k x", x=2),
                    in_=exv.rearrange("p (k x two) -> p k x two", x=2, two=2),
                    axis=AX.X, op=OP.add)
                ve.drain()
                # L2: add tree level 2 -> zs3 ; ae mult
                ve.tensor_tensor(
                    out=zs3, in0=zg_kqd[:, :, 0, :], in1=zg_kqd[:, :, 1, :],
                    op=OP.add).then_inc(s_dv)                    # SEL[i]: zg free
                ve.tensor_tensor(
                    out=exv[:, 0:2 * K].rearrange("p (k x) -> p k x", x=2),
                    in0=ex_v.rearrange("p (k x) -> p k x", x=2),
                    in1=bc_sb[:, C_WAE:C_WAE + 2].to_broadcast([128, 2, K])
                        .rearrange("p x k -> p k x"),
                    op=OP.mult)
                ve.drain()
                # L3: dot mult + ae reduce
                ve.tensor_tensor(
                    out=sc3, in0=zs3,
                    in1=bc_sb[:, C_WAS:C_WAS + 16].to_broadcast([128, 16, K])
                        .rearrange("p d k -> p k d"),
                    op=OP.mult)
                ve.tensor_reduce(
                    out=exv[:, 2 * K:3 * K],
                    in_=exv[:, 0:2 * K].rearrange("p (k x) -> p k x", x=2),
                    axis=AX.X, op=OP.add)
                ve.drain()
                # L4: dot reduce
                ve.tensor_reduce(out=sv_v, in_=sc3, axis=AX.X, op=OP.add)
                ve.drain()
                # L5: a = (sv + t_dst) + ae ; then + bias
                ve.scalar_tensor_tensor(
                    out=a_v, in0=sv_v, scalar=t_all[:, i:i + 1],
                    in1=exv[:, 2 * K:3 * K], op0=OP.add, op1=OP.add)
                ve.drain()
                ve.tensor_tensor(out=a_v, in0=a_v, in1=bias_v, op=OP.add)
                ve.drain()
                ve.scalar_tensor_tensor(
                    out=sv_v, in0=a_v, scalar=0.01,
                    in1=zcol[:].to_broadcast([128, K]),
                    op0=OP.mult, op1=OP.add)
                ve.drain()
                ve.tensor_tensor(out=a_v, in0=a_v, in1=sv_v,
                                 op=OP.max).then_inc(s_aw)
                # scalar engine computes w=exp(a), den
                ve.wait_ge(s_ac, i + 1)
                # L6: weighted sums
                ve.tensor_tensor(out=sc3, in0=zs3,
                                 in1=w_v.to_broadcast([128, K, 16]), op=OP.mult)
                ve.tensor_tensor(
                    out=exv[:, 0:2 * K].rearrange("p (k x) -> p k x", x=2),
                    in0=ex_v.rearrange("p (k x) -> p k x", x=2),
                    in1=w_v.to_broadcast([128, K, 2]), op=OP.mult)
                ve.scalar_tensor_tensor(
                    out=den, in0=den, scalar=1e-30,
                    in1=zcol[:, 0:1], op0=OP.add, op1=OP.add)
                ve.drain()
                ve.tensor_reduce(
                    out=obuf[:, 0:16],
                    in_=sc16[:, :16 * K].rearrange("p (k d) -> p d k", d=16),
                    axis=AX.X, op=OP.add)
                ve.tensor_reduce(
                    out=wex,
                    in_=exv[:, 0:2 * K].rearrange("p (k x) -> p x k", x=2),
                    axis=AX.X, op=OP.add)
                ve.reciprocal(rden, den)
                ve.drain()
                # L7: obuf = e2n0*wex0 + acc ; then e2n1*wex1 + ...
                ve.scalar_tensor_tensor(
                    out=obuf[:, 16:32], in0=bc_sb[:, C_E2N0:C_E2N0 + 16],
                    scalar=wex[:, 0:1], in1=obuf[:, 0:16],
                    op0=OP.mult, op1=OP.add)
                ve.drain()
                ve.scalar_tensor_tensor(
                    out=obuf[:, 0:16], in0=bc_sb[:, C_E2N1:C_E2N1 + 16],
                    scalar=wex[:, 1:2], in1=obuf[:, 16:32],
                    op0=OP.mult, op1=OP.add)
                if i >= 2:
                    ve.wait_ge(s_ot[b], 16 * (i // 2))
                ve.drain()
                ve.scalar_tensor_tensor(
                    out=otiles[b][:], in0=obuf[:, 0:16], scalar=rden,
                    in1=zcol[:].to_broadcast([128, 16]),
                    op0=OP.mult, op1=OP.add).then_inc(s_dv)      # OT[i]

    nc.compile()
    return nc


_CACHE = {}


def kernel(h, e, src, dst, W_fc, W_attn, W_edge, W_e2n):
    import concourse.bass_utils as bu

    h = np.asarray(h, np.float32)
    e = np.asarray(e, np.float32)
    src = np.asarray(src, np.int64)
    dst = np.asarray(dst, np.int64)
    W_fc = np.asarray(W_fc, np.float32)
    W_attn = np.asarray(W_attn, np.float32)
    W_edge = np.asarray(W_edge, np.float32)
    W_e2n = np.asarray(W_e2n, np.float32)

    K_sched, col_off, idx16, blob, hsh, node_at = _host_prep(h, e, src, dst)

    key = tuple(K_sched.tolist())
    if key not in _CACHE:
        _CACHE[key] = _build(K_sched, col_off)
    nc = _CACHE[key]

    aux = np.zeros((1, 72), np.float32)
    aux[0, 0:2] = W_edge[0, :]
    aux[0, 2:4] = W_edge[1, :]
    aux[0, 4:6] = W_attn[0, 2 * OUT_DIM:]
    aux[0, 6:22] = W_attn[0, :OUT_DIM]
    aux[0, 22:38] = W_e2n[:, 0]
    aux[0, 38:54] = W_e2n[:, 1]
    auxc = W_attn[0, OUT_DIM:2 * OUT_DIM].reshape(OUT_DIM, 1).astype(np.float32)

    in_maps = [{
        "hT": hsh[c], "wfc": W_fc, "aux": aux, "auxc": auxc,
        "idx16": idx16[c], "blob": blob[c],
    } for c in range(N_CORES)]
    trace = bool(int(os.environ.get("AGAT_TRACE", "0")))
    if trace:
        _install_ntff_shim()
    res = bu.run_bass_kernel_spmd(nc, in_maps, core_ids=list(range(N_CORES)),
                                  trace=trace)
    global LAST_EXEC_NS
    LAST_EXEC_NS = res.exec_time_ns

    out = np.zeros((N_NODES, OUT_DIM), np.float32)
    for c in range(N_CORES):
        oc = res.results[c]["out"].reshape(BLOCK, OUT_DIM)
        rows = node_at[c]
        v = rows >= 0
        out[rows[v]] = oc[v]
    return out


# revision 18
# speedup vs baseline: 1.0807x; 1.0712x over previous
"""AGAT layer (GNN message passing) on 8 TRN2 NeuronCores.

Strategy v2.2 (dst-sharded, padded-CSR, single collective, lean DVE):
  - Nodes degree-sorted into 128-node tiles; tile groups of 8 dealt
    round-robin to cores (identical SPMD schedule, balanced edges).
  - Each core computes z = h @ W_fc.T for its 12544-node block on PE
    (batched 4 tiles per matmul), packs 4 nodes per 256B row,
    AllGathers the table.
  - Per tile, 256B rows are fetched with dma_gather (int16 row idx);
    the 1-of-4 node select = qm mask-mult + 2-level add tree.
  - Attention: a_src dot on DVE, a_dst via per-tile PE matmul, leaky
    relu on DVE (STT ops only; tensor_scalar stalls erratically),
    exp + denominator accumulation on the Scalar engine (single
    activation table -> no reloads).
  - Stage B software-pipelined: PRE(i) interleaved with POST(i-1),
    drains only around the dependent attention tail (3 per tile).
  - dst nodes on partitions, edge slots on free dim; each core owns
    its dst nodes -> no accumulator all-reduce.
"""
import os
import numpy as np

LAST_EXEC_NS = None


def _install_ntff_shim():
    """Register the NTFF profile hook bass_utils expects under axon."""
    import sys
    import types
    import antenv
    if "antenv.axon_hooks" in sys.modules:
        return
    mod = types.ModuleType("antenv.axon_hooks")
    mod._hook = None
    mod.set_axon_ntff_profile_hook = lambda h: setattr(mod, "_hook", h)
    mod.get_axon_ntff_profile_hook = lambda: mod._hook
    sys.modules["antenv.axon_hooks"] = mod
    antenv.axon_hooks = mod
    try:
        from trn_agent_boot.trn_boot import _ntff_profile_via_ctypes
        mod.set_axon_ntff_profile_hook(
            _ntff_profile_via_ctypes("/opt/axon/libaxon_pjrt.so"))
    except Exception:
        pass


N_NODES = 100000
N_EDGES = 3200000
IN_DIM = 62
OUT_DIM = 16
EDGE_DIM = 2
N_CORES = 8
TILE = 128
TPC = 98                      # tiles per core
NT = N_CORES * TPC            # 784 tiles
NPAD = NT * TILE              # 100352 padded nodes
BLOCK = TPC * TILE            # 12544 nodes per core block
TROWS = NPAD // 4             # 25088 packed table rows
ROWW = 64                     # table row width in f32 (4 nodes x 16)
MAXC = 8                      # max k-columns per gather call -> NI <= 1024
GWIN = 8                      # outstanding gather calls
NEG_B01 = -60.0               # 0.01 * bias; a = 100*b01 + x, u = 0.01*x + b01
GRP = 4                       # tiles per z-matmul group (free dim 512)
NGRP = (TPC + GRP - 1) // GRP


def _host_prep(h, e, src, dst):
    deg = np.bincount(dst, minlength=N_NODES)
    order = np.argsort(-deg, kind="stable").astype(np.int64)
    order_pad = np.concatenate([order, np.full(NPAD - N_NODES, -1, np.int64)])

    degp = np.concatenate([deg, np.zeros(NPAD - N_NODES, np.int64)])
    deg_of = np.where(order_pad >= 0, degp[np.maximum(order_pad, 0)], 0)
    Kg = deg_of.reshape(NT, TILE).max(axis=1)
    K_sched = np.maximum(Kg.reshape(TPC, N_CORES).max(axis=1), 1).astype(np.int64)

    node_at = np.full((N_CORES, BLOCK), -1, np.int64)
    tabpos = np.full(N_NODES, -1, np.int64)
    for g in range(NT):
        i, c = divmod(g, N_CORES)
        nodes = order_pad[g * TILE:(g + 1) * TILE]
        node_at[c, i * TILE:(i + 1) * TILE] = nodes
        valid = nodes >= 0
        tp = c * BLOCK + i * TILE + np.nonzero(valid)[0]
        tabpos[nodes[valid]] = tp

    e_order = np.argsort(dst, kind="stable")
    csr_off = np.zeros(N_NODES + 1, np.int64)
    np.cumsum(deg, out=csr_off[1:])

    Ksum = int(K_sched.sum())
    col_off = np.zeros(TPC + 1, np.int64)
    np.cumsum(K_sched, out=col_off[1:])

    idx16 = np.zeros((N_CORES, 128, 16 * Ksum), np.int16)
    blob = np.zeros((N_CORES, 128, 9 * Ksum), np.float32)
    hsh = np.zeros((N_CORES, IN_DIM, BLOCK), np.float32)

    src_pos = tabpos[src]
    qrow_all = (src_pos // 4).astype(np.int32)
    qsel_all = (src_pos % 4).astype(np.int64)

    for c in range(N_CORES):
        hrows = node_at[c]
        hv = hrows >= 0
        hsh[c][:, hv] = h[hrows[hv]].T
        for i in range(TPC):
            K = int(K_sched[i])
            nodes = node_at[c, i * TILE:(i + 1) * TILE]
            co = int(col_off[i])
            eb = np.full((TILE, K), -1, np.int64)
            for p in range(TILE):
                n = nodes[p]
                if n < 0:
                    continue
                eds = e_order[csr_off[n]:csr_off[n + 1]]
                eb[p, :len(eds)] = eds
            vm = eb >= 0
            ebs = np.maximum(eb, 0)
            qi = np.where(vm, qrow_all[ebs], 0).astype(np.int16)  # [128, K]
            k0 = 0
            while k0 < K:
                Kc = min(MAXC, K - k0)
                NI = Kc * TILE
                jj = (np.arange(Kc)[:, None] * TILE + np.arange(TILE)[None, :])
                w16 = np.zeros((16, NI // 16), np.int16)
                w16[(jj % 16).ravel(), (jj // 16).ravel()] = qi[:, k0:k0 + Kc].T.ravel()
                cs = 16 * (co + k0)
                idx16[c][:, cs:cs + NI // 16] = np.tile(w16, (8, 1))
                k0 += Kc
            b0 = 9 * co
            ev = np.where(vm[:, :, None], e[ebs], 0.0).astype(np.float32)
            e4 = np.concatenate([ev, ev], axis=2)            # [128, K, 4]
            blob[c][:, b0:b0 + 4 * K] = e4.reshape(TILE, 4 * K)
            qm = np.zeros((TILE, K, 4), np.float32)
            pp, kk = np.nonzero(vm)
            qm[pp, kk, qsel_all[eb[pp, kk]]] = 1.0
            blob[c][:, b0 + 4 * K:b0 + 8 * K] = qm.reshape(TILE, 4 * K)
            blob[c][:, b0 + 8 * K:b0 + 9 * K] = np.where(vm, 0.0, NEG_B01)

    return K_sched, col_off, idx16, blob, hsh, node_at


def _build(K_sched, col_off):
    import concourse.bass as bass
    import concourse.bacc as bacc
    import concourse.mybir as mybir
    from concourse import library_config

    DT = mybir.dt.float32
    AF = mybir.ActivationFunctionType
    OP = mybir.AluOpType
    AX = mybir.AxisListType
    Ksum = int(K_sched.sum())
    KMAX = int(K_sched.max())
    NOCC = bool(int(os.environ.get("AGAT_NOCC", "0")))

    nc = bacc.Bacc(num_swdge_queues=4, dynamic_dma_scratch_size=32768)

    hT_ext = nc.declare_dram_parameter("hT", [IN_DIM, BLOCK], DT, isOutput=False)
    wfc_ext = nc.declare_dram_parameter("wfc", [OUT_DIM, IN_DIM], DT, isOutput=False)
    aux_ext = nc.declare_dram_parameter("aux", [1, 72], DT, isOutput=False)
    auxc_ext = nc.declare_dram_parameter("auxc", [OUT_DIM, 1], DT, isOutput=False)
    ident_ext = nc.declare_dram_parameter("ident", [128, 128], DT, isOutput=False)
    ones_ext = nc.declare_dram_parameter("ones", [1, 128], DT, isOutput=False)
    zcol_ext = nc.declare_dram_parameter("zcol", [128, 1], DT, isOutput=False)
    idx_ext = nc.declare_dram_parameter("idx16", [128, 16 * Ksum], mybir.dt.int16, isOutput=False)
    blob_ext = nc.declare_dram_parameter("blob", [128, 9 * Ksum], DT, isOutput=False)
    out_ext = nc.declare_dram_parameter("out", [TPC, 128, OUT_DIM], DT, isOutput=True)

    tabin = nc.dram_tensor("tabin" + os.environ.get("AGAT_V", ""), [BLOCK // 4, ROWW], DT)
    agtab = nc.dram_tensor("agtab", [TROWS, ROWW], DT, addr_space="Shared")

    # aux row layout (row-broadcast via ones-matmul into bc_sb)
    C_W4, C_WAE, C_WAS, C_E2N0, C_E2N1 = 0, 4, 6, 22, 38

    from contextlib import ExitStack
    with ExitStack() as _es:
        wfc_sb = _es.enter_context(nc.sbuf_tensor([16, IN_DIM], DT))
        wfcT_sb = _es.enter_context(nc.sbuf_tensor([IN_DIM, 16], DT))
        ones_sb = _es.enter_context(nc.sbuf_tensor([1, 128], DT))
        aux_sb = _es.enter_context(nc.sbuf_tensor([1, 72], DT))
        auxc_sb = _es.enter_context(nc.sbuf_tensor([16, 1], DT))
        bc_sb = _es.enter_context(nc.sbuf_tensor([128, 72], DT))
        ident_sb = _es.enter_context(nc.sbuf_tensor([128, 128], DT))
        zcol = _es.enter_context(nc.sbuf_tensor([128, 1], DT))
        t_all = _es.enter_context(nc.sbuf_tensor([128, TPC], DT))
        zT0 = _es.enter_context(nc.sbuf_tensor([16, GRP * 128], DT))
        zT1 = _es.enter_context(nc.sbuf_tensor([16, GRP * 128], DT))
        zpack = _es.enter_context(nc.sbuf_tensor([128, 8 * 16], DT))
        hT_sb = _es.enter_context(nc.sbuf_tensor([IN_DIM, BLOCK], DT))
        zg0 = _es.enter_context(nc.sbuf_tensor([128, KMAX * 64], DT))
        zg1 = _es.enter_context(nc.sbuf_tensor([128, KMAX * 64], DT))
        zg2 = _es.enter_context(nc.sbuf_tensor([128, KMAX * 64], DT))
        blob0 = _es.enter_context(nc.sbuf_tensor([128, KMAX * 9], DT))
        blob1 = _es.enter_context(nc.sbuf_tensor([128, KMAX * 9], DT))
        blob2 = _es.enter_context(nc.sbuf_tensor([128, KMAX * 9], DT))
        idx0 = _es.enter_context(nc.sbuf_tensor([128, KMAX * 16], mybir.dt.int16))
        idx1 = _es.enter_context(nc.sbuf_tensor([128, KMAX * 16], mybir.dt.int16))
        idx2 = _es.enter_context(nc.sbuf_tensor([128, KMAX * 16], mybir.dt.int16))
        zs0 = _es.enter_context(nc.sbuf_tensor([128, KMAX * 16], DT))
        zs1 = _es.enter_context(nc.sbuf_tensor([128, KMAX * 16], DT))
        sc160 = _es.enter_context(nc.sbuf_tensor([128, KMAX * 16], DT))
        sc161 = _es.enter_context(nc.sbuf_tensor([128, KMAX * 16], DT))
        wk0 = _es.enter_context(nc.sbuf_tensor([128, 10 * KMAX], DT))
        wk1 = _es.enter_context(nc.sbuf_tensor([128, 10 * KMAX], DT))
        smal0 = _es.enter_context(nc.sbuf_tensor([128, 8], DT))
        smal1 = _es.enter_context(nc.sbuf_tensor([128, 8], DT))
        obuf = _es.enter_context(nc.sbuf_tensor([128, 32], DT))
        otile0 = _es.enter_context(nc.sbuf_tensor([128, 16], DT))
        otile1 = _es.enter_context(nc.sbuf_tensor([128, 16], DT))
        ps_z0 = _es.enter_context(nc.psum_tensor([16, GRP * 128], DT))
        ps_z1 = _es.enter_context(nc.psum_tensor([16, GRP * 128], DT))
        ps_tr4 = _es.enter_context(nc.psum_tensor([128, GRP * 16], DT))
        ps_a4 = _es.enter_context(nc.psum_tensor([128, GRP], DT))
        ps_w = _es.enter_context(nc.psum_tensor([62, 16], DT))
        ps_bc = _es.enter_context(nc.psum_tensor([128, 72], DT))
        s_in = _es.enter_context(nc.semaphore("s_in"))
        s_pe = _es.enter_context(nc.semaphore("s_pe"))
        s_dv = _es.enter_context(nc.semaphore("s_dv"))
        s_ac = _es.enter_context(nc.semaphore("s_ac"))
        s_aw = _es.enter_context(nc.semaphore("s_aw"))
        s_gp = _es.enter_context(nc.semaphore("s_gp"))
        s_g = [_es.enter_context(nc.semaphore(f"s_g{q}")) for q in range(4)]
        s_cc = _es.enter_context(nc.semaphore("s_cc"))
        s_ot = [_es.enter_context(nc.semaphore(f"s_ot{p}")) for p in range(2)]
        s_tl = [_es.enter_context(nc.semaphore(f"s_tl{p}")) for p in range(3)]
        block = _es.enter_context(nc.Block())

        zgs, blobs, idxs = [zg0, zg1, zg2], [blob0, blob1, blob2], [idx0, idx1, idx2]
        zss, wks, smals = [zs0, zs1], [wk0, wk1], [smal0, smal1]
        sc16s = [sc160, sc161]
        zTs, ps_zs = [zT0, zT1], [ps_z0, ps_z1]
        otiles = [otile0, otile1]

        def tiles_of(g):
            return range(g * GRP, min((g + 1) * GRP, TPC))

        # ---------------- stage-A sem ledger ----------------
        pe_cnt = 2    # 1 ps_bc, 2 ps_w
        dv_cnt = 2    # 1 bc copy, 2 wfcT copy
        PE_Z = [0] * NGRP      # after ps_z matmul of group g
        PE_TRL = [0] * NGRP    # after last transpose of group g
        PE_AL = [0] * NGRP     # after last ps_a matmul of group g
        DV_ZT = [0] * NGRP     # after zT copy of group g
        DV_PK = [0] * NGRP     # after zpack copy of group g
        DV_TA = [0] * NGRP     # after t_all copy of group g
        for g in range(NGRP):
            nt = len(tiles_of(g))
            pe_cnt += 1
            PE_Z[g] = pe_cnt
            pe_cnt += nt
            PE_TRL[g] = pe_cnt
            pe_cnt += nt
            PE_AL[g] = pe_cnt
            dv_cnt += 1
            DV_ZT[g] = dv_cnt
            dv_cnt += 1
            DV_PK[g] = dv_cnt
            dv_cnt += 1
            DV_TA[g] = dv_cnt
        DVA_END = dv_cnt

        # stage-B ladder: iteration it emits SEL[it] (add2) then OT[it-1]
        SEL = [0] * TPC
        OT = [0] * TPC
        _c = DVA_END
        for it in range(TPC + 1):
            if it < TPC:
                _c += 1
                SEL[it] = _c
            if it >= 1:
                _c += 1
                OT[it - 1] = _c

        qcnt = [0, 0, 0, 0]
        qsnap = []
        call_hist = []

        def TL(i):
            return 32 * (i // 3 + 1)

        @block.sync
        def _(sy: bass.BassEngine):
            sy.dma_start(out=hT_sb[:], in_=hT_ext[:]).then_inc(s_in, 16)
            sy.dma_start(out=wfc_sb[:], in_=wfc_ext[:]).then_inc(s_in, 16)
            sy.dma_start(out=aux_sb[:], in_=aux_ext[:]).then_inc(s_in, 16)
            sy.dma_start(out=auxc_sb[:], in_=auxc_ext[:]).then_inc(s_in, 16)
            sy.dma_start(out=ident_sb[:], in_=ident_ext[:]).then_inc(s_in, 16)
            sy.dma_start(out=ones_sb[:], in_=ones_ext[:]).then_inc(s_in, 16)
            sy.dma_start(out=zcol[:], in_=zcol_ext[:]).then_inc(s_in, 16)
            for b in range(3):
                if b < TPC:
                    ko, K = int(col_off[b]), int(K_sched[b])
                    sy.dma_start(out=idxs[b][:, :16 * K],
                                 in_=idx_ext[:, 16 * ko:16 * (ko + K)]).then_inc(s_tl[b], 16)
                    sy.dma_start(out=blobs[b][:, :9 * K],
                                 in_=blob_ext[:, 9 * ko:9 * (ko + K)]).then_inc(s_tl[b], 16)
            for i in range(3, TPC + 3):
                if i < TPC:
                    sy.wait_ge(s_aw, i - 2)
                    K = int(K_sched[i])
                    co = int(col_off[i])
                    b = i % 3
                    sy.dma_start(out=idxs[b][:, :16 * K],
                                 in_=idx_ext[:, 16 * co:16 * (co + K)]).then_inc(s_tl[b], 16)
                    sy.dma_start(out=blobs[b][:, :9 * K],
                                 in_=blob_ext[:, 9 * co:9 * (co + K)]).then_inc(s_tl[b], 16)
                j = i - 3
                if 0 <= j < TPC:
                    sy.wait_ge(s_dv, OT[j])
                    sy.dma_start(out=out_ext[j],
                                 in_=otiles[j % 2][:]).then_inc(s_ot[j % 2], 16)

        @block.gpsimd
        def _(gp: bass.BassEngine):
            gp.load_library(library_config.mlp)
            # pack-group DMAs to tabin: every 8 tiles (2 stage-A groups)
            NPG = (TPC + 7) // 8
            for j in range(NPG):
                nch = min(8, TPC - 8 * j)
                lastg = (8 * j + nch - 1) // GRP
                gp.wait_ge(s_dv, DV_PK[lastg])
                gp.dma_start(
                    out=tabin[32 * 8 * j: 32 * 8 * j + 32 * nch, :].rearrange(
                        "(jj pp) (qq d) -> pp qq jj d", pp=32, qq=4),
                    in_=zpack[:, :nch * 16].rearrange("p (jj d) -> p jj d", d=16),
                ).then_inc(s_gp, 16)
            gp.wait_ge(s_gp, 16 * NPG)
            if NOCC:
                gp.dma_start(out=agtab[:BLOCK // 4, :], in_=tabin[:]).then_inc(s_cc, 16)
                gp.wait_ge(s_cc, 16)
            else:
                gp.collective_compute(
                    "AllGather", mybir.AluOpType.bypass,
                    replica_groups=[list(range(N_CORES))],
                    ins=[tabin[:]], outs=[agtab[:]],
                ).then_inc(s_cc)
                gp.wait_ge(s_cc, 1)
            call_no = 0
            for i in range(TPC):
                b = i % 3
                K = int(K_sched[i])
                gp.wait_ge(s_tl[b], TL(i))
                if i >= 3:
                    gp.wait_ge(s_dv, SEL[i - 3])
                k0 = 0
                while k0 < K:
                    Kc = min(MAXC, K - k0)
                    NI = Kc * TILE
                    q = call_no % 4
                    gp.dma_gather(
                        out_ap=zgs[b][:, 64 * k0:64 * (k0 + Kc)].rearrange(
                            "p (k w) -> p k w", w=64),
                        in_ap=agtab[:],
                        idxs_ap=idxs[b][:, 16 * k0:16 * k0 + NI // 16],
                        num_idxs=NI,
                        num_idxs_reg=NI,
                        elem_size=ROWW,
                        elem_step=ROWW,
                        queue_num=q,
                    ).then_inc(s_g[q], 16)
                    qcnt[q] += 1
                    call_hist.append((q, qcnt[q]))
                    call_no += 1
                    if len(call_hist) > GWIN:
                        oq, ocnt = call_hist[-(GWIN + 1)]
                        gp.wait_ge(s_g[oq], 16 * ocnt)
                    k0 += Kc
                qsnap.append(tuple(qcnt))

        @block.tensor
        def _(te: bass.BassEngine):
            te.wait_ge(s_in, 16 * 7)
            te.matmul(ps_bc[:], lhsT=ones_sb[:], rhs=aux_sb[:], start=True,
                      stop=True).then_inc(s_pe)                       # pe=1
            te.transpose(ps_w[:], in_=wfc_sb[:],
                         identity=ident_sb[:16, :16]).then_inc(s_pe)  # pe=2
            for g in range(NGRP):
                n = len(tiles_of(g)) * 128
                te.wait_ge(s_dv, 2 if g < 2 else DV_ZT[g - 2])
                te.matmul(ps_zs[g % 2][:, :n], lhsT=wfcT_sb[:],
                          rhs=hT_sb[:, g * GRP * 128:g * GRP * 128 + n],
                          start=True, stop=True).then_inc(s_pe)
                te.wait_ge(s_dv, DV_ZT[g])
                if g >= 1:
                    te.wait_ge(s_dv, DV_PK[g - 1])   # ps_tr4 free
                for t, i in enumerate(tiles_of(g)):
                    sl = zTs[g % 2][:, t * 128:t * 128 + 128]
                    te.transpose(ps_tr4[:, 16 * t:16 * t + 16], in_=sl,
                                 identity=ident_sb[:16, :16]).then_inc(s_pe)
                if g >= 1:
                    te.wait_ge(s_dv, DV_TA[g - 1])   # ps_a4 free
                for t, i in enumerate(tiles_of(g)):
                    sl = zTs[g % 2][:, t * 128:t * 128 + 128]
                    te.matmul(ps_a4[:, t:t + 1], lhsT=sl, rhs=auxc_sb[:],
                              start=True, stop=True).then_inc(s_pe)

        @block.scalar
        def _(sc: bass.BassEngine):
            for i in range(TPC):
                b = i % 2
                K = int(K_sched[i])
                a_v = wks[b][:, 0:K]
                w_v = wks[b][:, 2 * KMAX:2 * KMAX + K]
                sc.wait_ge(s_aw, i + 1)
                sc.activation(w_v, a_v, AF.Exp,
                              accum_out=smals[b][:, 0:1]).then_inc(s_ac)

        SAFE = int(os.environ.get("AGAT_SAFE", "0"))

        @block.vector
        def _(ve: bass.BassEngine):
            def sdrain(lvl=1):
                if SAFE >= lvl:
                    ve.drain()
            ve.wait_ge(s_pe, 1)
            ve.tensor_copy(bc_sb[:], ps_bc[:]).then_inc(s_dv)          # dv=1
            ve.wait_ge(s_pe, 2)
            ve.tensor_copy(wfcT_sb[:], ps_w[:]).then_inc(s_dv)         # dv=2
            # -------- stage A --------
            for g in range(NGRP):
                nt = len(tiles_of(g))
                ve.wait_ge(s_pe, PE_Z[g])
                ve.tensor_copy(zTs[g % 2][:, :nt * 128],
                               ps_zs[g % 2][:, :nt * 128]).then_inc(s_dv)
                ve.wait_ge(s_pe, PE_TRL[g])
                if g >= 2:
                    ve.wait_ge(s_gp, 16 * (g // 2))  # zpack half flushed
                ve.tensor_copy(zpack[:, 64 * (g % 2):64 * (g % 2) + 16 * nt],
                               ps_tr4[:, :16 * nt]).then_inc(s_dv)
                ve.wait_ge(s_pe, PE_AL[g])
                ve.tensor_copy(t_all[:, GRP * g:GRP * g + nt],
                               ps_a4[:, :nt]).then_inc(s_dv)
            # -------- stage B: PRE(it) interleaved with POST(it-1) --------
            for it in range(TPC + 1):
                T, P = it, it - 1
                if T < TPC:
                    bT, b3T, KT = T % 2, T % 3, int(K_sched[T])
                    zgT, blT, zsT = zgs[b3T], blobs[b3T], zss[bT]
                    e4T = blT[:, 0:4 * KT]
                    qmT = blT[:, 4 * KT:8 * KT]
                    b01T = blT[:, 8 * KT:9 * KT]
                    aT = wks[bT][:, 0:KT]
                    xT = wks[bT][:, KMAX:KMAX + KT]
                    # u reuses the ae slot (ae dead after the x STT)
                    uT = wks[bT][:, 7 * KMAX:7 * KMAX + KT]
                    exvT = wks[bT][:, 3 * KMAX:3 * KMAX + 4 * KT]
                    exT = wks[bT][:, 8 * KMAX:8 * KMAX + 2 * KT]
                    aeT = wks[bT][:, 7 * KMAX:7 * KMAX + KT]
                    zgf = zgT[:, :64 * KT].rearrange("p (kq d) -> p kq d", d=16)
                    zgh = zgT[:, :64 * KT].rearrange("p (k two qd) -> p k two qd",
                                                     two=2, qd=32)
                    zgq = zgT[:, :64 * KT].rearrange("p (k q d) -> p k q d",
                                                     q=4, d=16)
                    zs3T = zsT[:, :16 * KT].rearrange("p (k d) -> p k d", d=16)
                    sc3T = sc16s[bT][:, :16 * KT].rearrange("p (k d) -> p k d", d=16)
                if P >= 0:
                    bP, KP = P % 2, int(K_sched[P])
                    zsP = zss[bP]
                    wvP = wks[bP][:, 2 * KMAX:2 * KMAX + KP]
                    exvP = wks[bP][:, 3 * KMAX:3 * KMAX + 4 * KP]
                    exP = wks[bP][:, 8 * KMAX:8 * KMAX + 2 * KP]
                    zs3P = zsP[:, :16 * KP].rearrange("p (k d) -> p k d", d=16)
                    sc3P = sc16s[bP][:, :16 * KP].rearrange("p (k d) -> p k d",
                                                            d=16)
                    denP, rdenP, wexP = (smals[bP][:, 0:1], smals[bP][:, 1:2],
                                         smals[bP][:, 2:4])

                # ---- L1 ----
                if T < TPC:
                    for q in range(4):
                        if qsnap[T][q] > 0:
                            ve.wait_ge(s_g[q], 16 * qsnap[T][q])
                    ve.wait_ge(s_tl[b3T], TL(T))
                    ve.tensor_tensor(out=zgf, in0=zgf,
                                     in1=qmT.to_broadcast([128, 4 * KT, 16]),
                                     op=OP.mult)
                if P >= 0:
                    ve.wait_ge(s_ac, P + 1)
                    ve.tensor_tensor(out=sc3P, in0=zs3P,
                                     in1=wvP.to_broadcast([128, KP, 16]),
                                     op=OP.mult)
                    ve.tensor_tensor(
                        out=exvP[:, 0:2 * KP].rearrange("p (k x) -> p k x", x=2),
                        in0=exP.rearrange("p (k x) -> p k x", x=2),
                        in1=wvP.to_broadcast([128, KP, 2]), op=OP.mult)
                ve.drain()
                # ---- L2 ----
                if T < TPC:
                    ve.tensor_tensor(out=zgh[:, :, 0, :], in0=zgh[:, :, 0, :],
                                     in1=zgh[:, :, 1, :], op=OP.add)
                if P >= 0:
                    ve.scalar_tensor_tensor(out=denP, in0=denP, scalar=1e-30,
                                            in1=zcol[:, 0:1], op0=OP.add,
                                            op1=OP.add)
                    ve.tensor_reduce(
                        out=obuf[:, 0:16],
                        in_=sc16s[bP][:, :16 * KP].rearrange(
                            "p (k d) -> p d k", d=16),
                        axis=AX.X, op=OP.add)
                if T < TPC:
                    ve.tensor_tensor(
                        out=exvT.rearrange("p (k x) -> p k x", x=4),
                        in0=e4T.rearrange("p (k x) -> p k x", x=4),
                        in1=bc_sb[:, C_W4:C_W4 + 4].to_broadcast([128, 4, KT])
                            .rearrange("p x k -> p k x"),
                        op=OP.mult)
                ve.drain()
                # ---- L3 ----
                if T < TPC:
                    ve.tensor_tensor(out=zs3T, in0=zgq[:, :, 0, :],
                                     in1=zgq[:, :, 1, :],
                                     op=OP.add).then_inc(s_dv)      # SEL[T]
                if P >= 0:
                    ve.tensor_reduce(
                        out=wexP,
                        in_=exvP[:, 0:2 * KP].rearrange("p (k x) -> p x k", x=2),
                        axis=AX.X, op=OP.add)
                    ve.reciprocal(rdenP, denP)
                if T < TPC:
                    ve.tensor_reduce(
                        out=exT.rearrange("p (k x) -> p k x", x=2),
                        in_=exvT.rearrange("p (k x two) -> p k x two", x=2, two=2),
                        axis=AX.X, op=OP.add)
                ve.drain()
                # ---- L4 ----
                if T < TPC:
                    ve.tensor_tensor(
                        out=sc3T, in0=zs3T,
                        in1=bc_sb[:, C_WAS:C_WAS + 16].to_broadcast([128, 16, KT])
                            .rearrange("p d k -> p k d"),
                        op=OP.mult)
                if P >= 0:
                    ve.scalar_tensor_tensor(
                        out=obuf[:, 16:32], in0=bc_sb[:, C_E2N0:C_E2N0 + 16],
                        scalar=wexP[:, 0:1], in1=obuf[:, 0:16],
                        op0=OP.mult, op1=OP.add)
                if T < TPC:
                    ve.tensor_tensor(
                        out=exvT[:, 0:2 * KT].rearrange("p (k x) -> p k x", x=2),
                        in0=exT.rearrange("p (k x) -> p k x", x=2),
                        in1=bc_sb[:, C_WAE:C_WAE + 2].to_broadcast([128, 2, KT])
                            .rearrange("p x k -> p k x"),
                        op=OP.mult)
                ve.drain()
                # ---- L5 ----
                if T < TPC:
                    ve.tensor_reduce(out=xT, in_=sc3T, axis=AX.X, op=OP.add)
                if P >= 0:
                    ve.scalar_tensor_tensor(
                        out=obuf[:, 0:16], in0=bc_sb[:, C_E2N1:C_E2N1 + 16],
                        scalar=wexP[:, 1:2], in1=obuf[:, 16:32],
                        op0=OP.mult, op1=OP.add)
                if T < TPC:
                    ve.tensor_reduce(
                        out=aeT,
                        in_=exvT[:, 0:2 * KT].rearrange("p (k x) -> p k x", x=2),
                        axis=AX.X, op=OP.add)
                ve.drain()
                # ---- L6 ----
                if T < TPC:
                    ve.scalar_tensor_tensor(
                        out=xT, in0=xT, scalar=t_all[:, T:T + 1], in1=aeT,
                        op0=OP.add, op1=OP.add)
                if P >= 0:
                    if P >= 2:
                        ve.wait_ge(s_ot[bP], 16 * (P // 2))
                    ve.scalar_tensor_tensor(
                        out=otiles[bP][:], in0=obuf[:, 0:16], scalar=rdenP,
                        in1=zcol[:].to_broadcast([128, 16]),
                        op0=OP.mult, op1=OP.add).then_inc(s_dv)     # OT[P]
                ve.drain()
                # ---- L7 ----
                if T < TPC:
                    ve.scalar_tensor_tensor(
                        out=aT, in0=b01T, scalar=100.0, in1=xT,
                        op0=OP.mult, op1=OP.add)
                    ve.scalar_tensor_tensor(
                        out=uT, in0=xT, scalar=0.01, in1=b01T,
                        op0=OP.mult, op1=OP.add)
                    ve.drain()
                    # ---- L8 ----
                    ve.tensor_tensor(out=aT, in0=aT, in1=uT,
                                     op=OP.max).then_inc(s_aw)

    nc.compile()
    return nc


_CACHE = {}


def kernel(h, e, src, dst, W_fc, W_attn, W_edge, W_e2n):
    import concourse.bass_utils as bu

    h = np.asarray(h, np.float32)
    e = np.asarray(e, np.float32)
    src = np.asarray(src, np.int64)
    dst = np.asarray(dst, np.int64)
    W_fc = np.asarray(W_fc, np.float32)
    W_attn = np.asarray(W_attn, np.float32)
    W_edge = np.asarray(W_edge, np.float32)
    W_e2n = np.asarray(W_e2n, np.float32)

    K_sched, col_off, idx16, blob, hsh, node_at = _host_prep(h, e, src, dst)

    key = tuple(K_sched.tolist())
    if key not in _CACHE:
        _CACHE[key] = _build(K_sched, col_off)
    nc = _CACHE[key]

    aux = np.zeros((1, 72), np.float32)
    aux[0, 0:2] = W_edge[0, :]
    aux[0, 2:4] = W_edge[1, :]
    aux[0, 4:6] = W_attn[0, 2 * OUT_DIM:]
    aux[0, 6:22] = W_attn[0, :OUT_DIM]
    aux[0, 22:38] = W_e2n[:, 0]
    aux[0, 38:54] = W_e2n[:, 1]
    auxc = W_attn[0, OUT_DIM:2 * OUT_DIM].reshape(OUT_DIM, 1).astype(np.float32)
    ident = np.eye(128, dtype=np.float32)
    ones = np.ones((1, 128), np.float32)
    zcol = np.zeros((128, 1), np.float32)

    in_maps = [{
        "hT": hsh[c], "wfc": W_fc, "aux": aux, "auxc": auxc,
        "ident": ident, "ones": ones, "zcol": zcol,
        "idx16": idx16[c], "blob": blob[c],
    } for c in range(N_CORES)]
    trace = bool(int(os.environ.get("AGAT_TRACE", "0")))
    if trace:
        _install_ntff_shim()
    res = bu.run_bass_kernel_spmd(nc, in_maps, core_ids=list(range(N_CORES)),
                                  trace=trace)
    global LAST_EXEC_NS
    LAST_EXEC_NS = res.exec_time_ns

    out = np.zeros((N_NODES, OUT_DIM), np.float32)
    for c in range(N_CORES):
        oc = res.results[c]["out"].reshape(BLOCK, OUT_DIM)
        rows = node_at[c]
        v = rows >= 0
        out[rows[v]] = oc[v]
    return out


# revision 19
# speedup vs baseline: 1.0864x; 1.0053x over previous
"""AGAT layer (GNN message passing) on 8 TRN2 NeuronCores.

Strategy v2.2 (dst-sharded, padded-CSR, single collective, lean DVE):
  - Nodes degree-sorted into 128-node tiles; tile groups of 8 dealt
    round-robin to cores (identical SPMD schedule, balanced edges).
  - Each core computes z = h @ W_fc.T for its 12544-node block on PE
    (batched 4 tiles per matmul), packs 4 nodes per 256B row,
    AllGathers the table.
  - Per tile, 256B rows are fetched with dma_gather (int16 row idx);
    the 1-of-4 node select = qm mask-mult + 2-level add tree.
  - Attention: a_src dot on DVE, a_dst via per-tile PE matmul, leaky
    relu on DVE (STT ops only; tensor_scalar stalls erratically),
    exp + denominator accumulation on the Scalar engine (single
    activation table -> no reloads).
  - Stage B software-pipelined: PRE(i) interleaved with POST(i-1),
    drains only around the dependent attention tail (3 per tile).
  - dst nodes on partitions, edge slots on free dim; each core owns
    its dst nodes -> no accumulator all-reduce.
"""
import os
import numpy as np

LAST_EXEC_NS = None


def _install_ntff_shim():
    """Register the NTFF profile hook bass_utils expects under axon."""
    import sys
    import types
    import antenv
    if "antenv.axon_hooks" in sys.modules:
        return
    mod = types.ModuleType("antenv.axon_hooks")
    mod._hook = None
    mod.set_axon_ntff_profile_hook = lambda h: setattr(mod, "_hook", h)
    mod.get_axon_ntff_profile_hook = lambda: mod._hook
    sys.modules["antenv.axon_hooks"] = mod
    antenv.axon_hooks = mod
    try:
        from trn_agent_boot.trn_boot import _ntff_profile_via_ctypes
        mod.set_axon_ntff_profile_hook(
            _ntff_profile_via_ctypes("/opt/axon/libaxon_pjrt.so"))
    except Exception:
        pass


N_NODES = 100000
N_EDGES = 3200000
IN_DIM = 62
OUT_DIM = 16
EDGE_DIM = 2
N_CORES = 8
TILE = 128
TPC = 98                      # tiles per core
NT = N_CORES * TPC            # 784 tiles
NPAD = NT * TILE              # 100352 padded nodes
BLOCK = TPC * TILE            # 12544 nodes per core block
TROWS = NPAD // 4             # 25088 packed table rows
ROWW = 64                     # table row width in f32 (4 nodes x 16)
MAXC = 8                      # max k-columns per gather call -> NI <= 1024
GWIN = 8                      # outstanding gather calls
NEG_B01 = -60.0               # 0.01 * bias; a = 100*b01 + x, u = 0.01*x + b01
GRP = 4                       # tiles per z-matmul group (free dim 512)
NGRP = (TPC + GRP - 1) // GRP


def _host_prep(h, e, src, dst):
    deg = np.bincount(dst, minlength=N_NODES)
    order = np.argsort(-deg, kind="stable").astype(np.int64)
    order_pad = np.concatenate([order, np.full(NPAD - N_NODES, -1, np.int64)])

    degp = np.concatenate([deg, np.zeros(NPAD - N_NODES, np.int64)])
    deg_of = np.where(order_pad >= 0, degp[np.maximum(order_pad, 0)], 0)
    Kg = deg_of.reshape(NT, TILE).max(axis=1)
    K_sched = np.maximum(Kg.reshape(TPC, N_CORES).max(axis=1), 1).astype(np.int64)

    node_at = np.full((N_CORES, BLOCK), -1, np.int64)
    tabpos = np.full(N_NODES, -1, np.int64)
    for g in range(NT):
        i, c = divmod(g, N_CORES)
        nodes = order_pad[g * TILE:(g + 1) * TILE]
        node_at[c, i * TILE:(i + 1) * TILE] = nodes
        valid = nodes >= 0
        tp = c * BLOCK + i * TILE + np.nonzero(valid)[0]
        tabpos[nodes[valid]] = tp

    e_order = np.argsort(dst, kind="stable")
    csr_off = np.zeros(N_NODES + 1, np.int64)
    np.cumsum(deg, out=csr_off[1:])

    Ksum = int(K_sched.sum())
    col_off = np.zeros(TPC + 1, np.int64)
    np.cumsum(K_sched, out=col_off[1:])

    idx16 = np.zeros((N_CORES, 128, 16 * Ksum), np.int16)
    blob = np.zeros((N_CORES, 128, 9 * Ksum), np.float32)
    hsh = np.zeros((N_CORES, IN_DIM, BLOCK), np.float32)

    src_pos = tabpos[src]
    qrow_all = (src_pos // 4).astype(np.int32)
    qsel_all = (src_pos % 4).astype(np.int64)

    for c in range(N_CORES):
        hrows = node_at[c]
        hv = hrows >= 0
        hsh[c][:, hv] = h[hrows[hv]].T
        for i in range(TPC):
            K = int(K_sched[i])
            nodes = node_at[c, i * TILE:(i + 1) * TILE]
            co = int(col_off[i])
            eb = np.full((TILE, K), -1, np.int64)
            for p in range(TILE):
                n = nodes[p]
                if n < 0:
                    continue
                eds = e_order[csr_off[n]:csr_off[n + 1]]
                eb[p, :len(eds)] = eds
            vm = eb >= 0
            ebs = np.maximum(eb, 0)
            qi = np.where(vm, qrow_all[ebs], 0).astype(np.int16)  # [128, K]
            k0 = 0
            while k0 < K:
                Kc = min(MAXC, K - k0)
                NI = Kc * TILE
                jj = (np.arange(Kc)[:, None] * TILE + np.arange(TILE)[None, :])
                w16 = np.zeros((16, NI // 16), np.int16)
                w16[(jj % 16).ravel(), (jj // 16).ravel()] = qi[:, k0:k0 + Kc].T.ravel()
                cs = 16 * (co + k0)
                idx16[c][:, cs:cs + NI // 16] = np.tile(w16, (8, 1))
                k0 += Kc
            b0 = 9 * co
            ev = np.where(vm[:, :, None], e[ebs], 0.0).astype(np.float32)
            e4 = np.concatenate([ev, ev], axis=2)            # [128, K, 4]
            blob[c][:, b0:b0 + 4 * K] = e4.reshape(TILE, 4 * K)
            qm = np.zeros((TILE, K, 4), np.float32)
            pp, kk = np.nonzero(vm)
            qm[pp, kk, qsel_all[eb[pp, kk]]] = 1.0
            blob[c][:, b0 + 4 * K:b0 + 8 * K] = qm.reshape(TILE, 4 * K)
            blob[c][:, b0 + 8 * K:b0 + 9 * K] = np.where(vm, 0.0, NEG_B01)

    return K_sched, col_off, idx16, blob, hsh, node_at


def _build(K_sched, col_off):
    import concourse.bass as bass
    import concourse.bacc as bacc
    import concourse.mybir as mybir
    from concourse import library_config

    DT = mybir.dt.float32
    AF = mybir.ActivationFunctionType
    OP = mybir.AluOpType
    AX = mybir.AxisListType
    Ksum = int(K_sched.sum())
    KMAX = int(K_sched.max())
    NOCC = bool(int(os.environ.get("AGAT_NOCC", "0")))

    nc = bacc.Bacc(num_swdge_queues=4, dynamic_dma_scratch_size=32768)

    hT_ext = nc.declare_dram_parameter("hT", [IN_DIM, BLOCK], DT, isOutput=False)
    wfc_ext = nc.declare_dram_parameter("wfc", [OUT_DIM, IN_DIM], DT, isOutput=False)
    aux_ext = nc.declare_dram_parameter("aux", [1, 72], DT, isOutput=False)
    auxc_ext = nc.declare_dram_parameter("auxc", [OUT_DIM, 1], DT, isOutput=False)
    ident_ext = nc.declare_dram_parameter("ident", [128, 128], DT, isOutput=False)
    ones_ext = nc.declare_dram_parameter("ones", [1, 128], DT, isOutput=False)
    zcol_ext = nc.declare_dram_parameter("zcol", [128, 1], DT, isOutput=False)
    idx_ext = nc.declare_dram_parameter("idx16", [128, 16 * Ksum], mybir.dt.int16, isOutput=False)
    blob_ext = nc.declare_dram_parameter("blob", [128, 9 * Ksum], DT, isOutput=False)
    out_ext = nc.declare_dram_parameter("out", [TPC, 128, OUT_DIM], DT, isOutput=True)

    tabin = nc.dram_tensor("tabin" + os.environ.get("AGAT_V", ""), [BLOCK // 4, ROWW], DT)
    agtab = nc.dram_tensor("agtab", [TROWS, ROWW], DT, addr_space="Shared")
    fence = nc.dram_tensor("ccfence", [N_CORES, ROWW], DT, addr_space="Shared")

    # aux row layout (row-broadcast via ones-matmul into bc_sb)
    C_W4, C_WAE, C_WAS, C_E2N0, C_E2N1 = 0, 4, 6, 22, 38

    from contextlib import ExitStack
    with ExitStack() as _es:
        wfc_sb = _es.enter_context(nc.sbuf_tensor([16, IN_DIM], DT))
        wfcT_sb = _es.enter_context(nc.sbuf_tensor([IN_DIM, 16], DT))
        ones_sb = _es.enter_context(nc.sbuf_tensor([1, 128], DT))
        aux_sb = _es.enter_context(nc.sbuf_tensor([1, 72], DT))
        auxc_sb = _es.enter_context(nc.sbuf_tensor([16, 1], DT))
        bc_sb = _es.enter_context(nc.sbuf_tensor([128, 72], DT))
        ident_sb = _es.enter_context(nc.sbuf_tensor([128, 128], DT))
        zcol = _es.enter_context(nc.sbuf_tensor([128, 1], DT))
        t_all = _es.enter_context(nc.sbuf_tensor([128, TPC], DT))
        zT0 = _es.enter_context(nc.sbuf_tensor([16, GRP * 128], DT))
        zT1 = _es.enter_context(nc.sbuf_tensor([16, GRP * 128], DT))
        zpack = _es.enter_context(nc.sbuf_tensor([128, 8 * 16], DT))
        hT_sb = _es.enter_context(nc.sbuf_tensor([IN_DIM, BLOCK], DT))
        zg0 = _es.enter_context(nc.sbuf_tensor([128, KMAX * 64], DT))
        zg1 = _es.enter_context(nc.sbuf_tensor([128, KMAX * 64], DT))
        zg2 = _es.enter_context(nc.sbuf_tensor([128, KMAX * 64], DT))
        blob0 = _es.enter_context(nc.sbuf_tensor([128, KMAX * 9], DT))
        blob1 = _es.enter_context(nc.sbuf_tensor([128, KMAX * 9], DT))
        blob2 = _es.enter_context(nc.sbuf_tensor([128, KMAX * 9], DT))
        idx0 = _es.enter_context(nc.sbuf_tensor([128, KMAX * 16], mybir.dt.int16))
        idx1 = _es.enter_context(nc.sbuf_tensor([128, KMAX * 16], mybir.dt.int16))
        idx2 = _es.enter_context(nc.sbuf_tensor([128, KMAX * 16], mybir.dt.int16))
        zs0 = _es.enter_context(nc.sbuf_tensor([128, KMAX * 16], DT))
        zs1 = _es.enter_context(nc.sbuf_tensor([128, KMAX * 16], DT))
        sc160 = _es.enter_context(nc.sbuf_tensor([128, KMAX * 16], DT))
        sc161 = _es.enter_context(nc.sbuf_tensor([128, KMAX * 16], DT))
        wk0 = _es.enter_context(nc.sbuf_tensor([128, 10 * KMAX], DT))
        wk1 = _es.enter_context(nc.sbuf_tensor([128, 10 * KMAX], DT))
        smal0 = _es.enter_context(nc.sbuf_tensor([128, 8], DT))
        smal1 = _es.enter_context(nc.sbuf_tensor([128, 8], DT))
        obuf = _es.enter_context(nc.sbuf_tensor([128, 32], DT))
        otile0 = _es.enter_context(nc.sbuf_tensor([128, 16], DT))
        otile1 = _es.enter_context(nc.sbuf_tensor([128, 16], DT))
        ps_z0 = _es.enter_context(nc.psum_tensor([16, GRP * 128], DT))
        ps_z1 = _es.enter_context(nc.psum_tensor([16, GRP * 128], DT))
        ps_tr4 = _es.enter_context(nc.psum_tensor([128, GRP * 16], DT))
        ps_a4 = _es.enter_context(nc.psum_tensor([128, GRP], DT))
        ps_w = _es.enter_context(nc.psum_tensor([62, 16], DT))
        ps_bc = _es.enter_context(nc.psum_tensor([128, 72], DT))
        s_in = _es.enter_context(nc.semaphore("s_in"))
        s_pe = _es.enter_context(nc.semaphore("s_pe"))
        s_dv = _es.enter_context(nc.semaphore("s_dv"))
        s_ac = _es.enter_context(nc.semaphore("s_ac"))
        s_aw = _es.enter_context(nc.semaphore("s_aw"))
        s_gp = _es.enter_context(nc.semaphore("s_gp"))
        s_g = [_es.enter_context(nc.semaphore(f"s_g{q}")) for q in range(4)]
        s_cc = _es.enter_context(nc.semaphore("s_cc"))
        s_ot = [_es.enter_context(nc.semaphore(f"s_ot{p}")) for p in range(2)]
        s_tl = [_es.enter_context(nc.semaphore(f"s_tl{p}")) for p in range(3)]
        block = _es.enter_context(nc.Block())

        zgs, blobs, idxs = [zg0, zg1, zg2], [blob0, blob1, blob2], [idx0, idx1, idx2]
        zss, wks, smals = [zs0, zs1], [wk0, wk1], [smal0, smal1]
        sc16s = [sc160, sc161]
        zTs, ps_zs = [zT0, zT1], [ps_z0, ps_z1]
        otiles = [otile0, otile1]

        def tiles_of(g):
            return range(g * GRP, min((g + 1) * GRP, TPC))

        # ---------------- stage-A sem ledger ----------------
        pe_cnt = 2    # 1 ps_bc, 2 ps_w
        dv_cnt = 2    # 1 bc copy, 2 wfcT copy
        PE_Z = [0] * NGRP      # after ps_z matmul of group g
        PE_TRL = [0] * NGRP    # after last transpose of group g
        PE_AL = [0] * NGRP     # after last ps_a matmul of group g
        DV_ZT = [0] * NGRP     # after zT copy of group g
        DV_PK = [0] * NGRP     # after zpack copy of group g
        DV_TA = [0] * NGRP     # after t_all copy of group g
        for g in range(NGRP):
            nt = len(tiles_of(g))
            pe_cnt += 1
            PE_Z[g] = pe_cnt
            pe_cnt += nt
            PE_TRL[g] = pe_cnt
            pe_cnt += nt
            PE_AL[g] = pe_cnt
            dv_cnt += 1
            DV_ZT[g] = dv_cnt
            dv_cnt += 1
            DV_PK[g] = dv_cnt
            dv_cnt += 1
            DV_TA[g] = dv_cnt
        DVA_END = dv_cnt

        # stage-B ladder: iteration it emits SEL[it] (add2) then OT[it-1]
        SEL = [0] * TPC
        OT = [0] * TPC
        _c = DVA_END
        for it in range(TPC + 1):
            if it < TPC:
                _c += 1
                SEL[it] = _c
            if it >= 1:
                _c += 1
                OT[it - 1] = _c

        qcnt = [0, 0, 0, 0]
        qsnap = []
        call_hist = []

        def TL(i):
            return 32 * (i // 3 + 1)

        @block.sync
        def _(sy: bass.BassEngine):
            sy.dma_start(out=hT_sb[:], in_=hT_ext[:]).then_inc(s_in, 16)
            sy.dma_start(out=wfc_sb[:], in_=wfc_ext[:]).then_inc(s_in, 16)
            sy.dma_start(out=aux_sb[:], in_=aux_ext[:]).then_inc(s_in, 16)
            sy.dma_start(out=auxc_sb[:], in_=auxc_ext[:]).then_inc(s_in, 16)
            sy.dma_start(out=ident_sb[:], in_=ident_ext[:]).then_inc(s_in, 16)
            sy.dma_start(out=ones_sb[:], in_=ones_ext[:]).then_inc(s_in, 16)
            sy.dma_start(out=zcol[:], in_=zcol_ext[:]).then_inc(s_in, 16)
            for b in range(3):
                if b < TPC:
                    ko, K = int(col_off[b]), int(K_sched[b])
                    sy.dma_start(out=idxs[b][:, :16 * K],
                                 in_=idx_ext[:, 16 * ko:16 * (ko + K)]).then_inc(s_tl[b], 16)
                    sy.dma_start(out=blobs[b][:, :9 * K],
                                 in_=blob_ext[:, 9 * ko:9 * (ko + K)]).then_inc(s_tl[b], 16)
            for i in range(3, TPC + 3):
                if i < TPC:
                    sy.wait_ge(s_aw, i - 2)
                    K = int(K_sched[i])
                    co = int(col_off[i])
                    b = i % 3
                    sy.dma_start(out=idxs[b][:, :16 * K],
                                 in_=idx_ext[:, 16 * co:16 * (co + K)]).then_inc(s_tl[b], 16)
                    sy.dma_start(out=blobs[b][:, :9 * K],
                                 in_=blob_ext[:, 9 * co:9 * (co + K)]).then_inc(s_tl[b], 16)
                j = i - 3
                if 0 <= j < TPC:
                    sy.wait_ge(s_dv, OT[j])
                    sy.dma_start(out=out_ext[j],
                                 in_=otiles[j % 2][:]).then_inc(s_ot[j % 2], 16)

        @block.gpsimd
        def _(gp: bass.BassEngine):
            gp.load_library(library_config.mlp)
            # pack-group DMAs to tabin: every 8 tiles (2 stage-A groups)
            NPG = (TPC + 7) // 8
            for j in range(NPG):
                nch = min(8, TPC - 8 * j)
                lastg = (8 * j + nch - 1) // GRP
                gp.wait_ge(s_dv, DV_PK[lastg])
                gp.dma_start(
                    out=tabin[32 * 8 * j: 32 * 8 * j + 32 * nch, :].rearrange(
                        "(jj pp) (qq d) -> pp qq jj d", pp=32, qq=4),
                    in_=zpack[:, :nch * 16].rearrange("p (jj d) -> p jj d", d=16),
                ).then_inc(s_gp, 16)
            gp.wait_ge(s_gp, 16 * NPG)
            if NOCC:
                gp.dma_start(out=agtab[:BLOCK // 4, :], in_=tabin[:]).then_inc(s_cc, 16)
                gp.wait_ge(s_cc, 16)
            else:
                gp.collective_compute(
                    "AllGather", mybir.AluOpType.bypass,
                    replica_groups=[list(range(N_CORES))],
                    ins=[tabin[:]], outs=[agtab[:]],
                ).then_inc(s_cc)
                # tiny fence collective: NRT runs collectives in order, so
                # its completion implies the big AllGather's data landed
                gp.collective_compute(
                    "AllGather", mybir.AluOpType.bypass,
                    replica_groups=[list(range(N_CORES))],
                    ins=[tabin[0:1, :]], outs=[fence[:]],
                ).then_inc(s_cc)
                gp.wait_ge(s_cc, 2)
            call_no = 0
            for i in range(TPC):
                b = i % 3
                K = int(K_sched[i])
                gp.wait_ge(s_tl[b], TL(i))
                if i >= 3:
                    gp.wait_ge(s_dv, SEL[i - 3])
                k0 = 0
                while k0 < K:
                    Kc = min(MAXC, K - k0)
                    NI = Kc * TILE
                    q = call_no % 4
                    gp.dma_gather(
                        out_ap=zgs[b][:, 64 * k0:64 * (k0 + Kc)].rearrange(
                            "p (k w) -> p k w", w=64),
                        in_ap=agtab[:],
                        idxs_ap=idxs[b][:, 16 * k0:16 * k0 + NI // 16],
                        num_idxs=NI,
                        num_idxs_reg=NI,
                        elem_size=ROWW,
                        elem_step=ROWW,
                        queue_num=q,
                    ).then_inc(s_g[q], 16)
                    qcnt[q] += 1
                    call_hist.append((q, qcnt[q]))
                    call_no += 1
                    if len(call_hist) > GWIN:
                        oq, ocnt = call_hist[-(GWIN + 1)]
                        gp.wait_ge(s_g[oq], 16 * ocnt)
                    k0 += Kc
                qsnap.append(tuple(qcnt))

        @block.tensor
        def _(te: bass.BassEngine):
            te.wait_ge(s_in, 16 * 7)
            te.matmul(ps_bc[:], lhsT=ones_sb[:], rhs=aux_sb[:], start=True,
                      stop=True).then_inc(s_pe)                       # pe=1
            te.transpose(ps_w[:], in_=wfc_sb[:],
                         identity=ident_sb[:16, :16]).then_inc(s_pe)  # pe=2
            for g in range(NGRP):
                n = len(tiles_of(g)) * 128
                te.wait_ge(s_dv, 2 if g < 2 else DV_ZT[g - 2])
                te.matmul(ps_zs[g % 2][:, :n], lhsT=wfcT_sb[:],
                          rhs=hT_sb[:, g * GRP * 128:g * GRP * 128 + n],
                          start=True, stop=True).then_inc(s_pe)
                te.wait_ge(s_dv, DV_ZT[g])
                if g >= 1:
                    te.wait_ge(s_dv, DV_PK[g - 1])   # ps_tr4 free
                for t, i in enumerate(tiles_of(g)):
                    sl = zTs[g % 2][:, t * 128:t * 128 + 128]
                    te.transpose(ps_tr4[:, 16 * t:16 * t + 16], in_=sl,
                                 identity=ident_sb[:16, :16]).then_inc(s_pe)
                if g >= 1:
                    te.wait_ge(s_dv, DV_TA[g - 1])   # ps_a4 free
                for t, i in enumerate(tiles_of(g)):
                    sl = zTs[g % 2][:, t * 128:t * 128 + 128]
                    te.matmul(ps_a4[:, t:t + 1], lhsT=sl, rhs=auxc_sb[:],
                              start=True, stop=True).then_inc(s_pe)

        @block.scalar
        def _(sc: bass.BassEngine):
            for i in range(TPC):
                b = i % 2
                K = int(K_sched[i])
                a_v = wks[b][:, 0:K]
                w_v = wks[b][:, 2 * KMAX:2 * KMAX + K]
                sc.wait_ge(s_aw, i + 1)
                sc.activation(w_v, a_v, AF.Exp,
                              accum_out=smals[b][:, 0:1]).then_inc(s_ac)

        SAFE = int(os.environ.get("AGAT_SAFE", "0"))

        @block.vector
        def _(ve: bass.BassEngine):
            def sdrain(lvl=1):
                if SAFE >= lvl:
                    ve.drain()
            ve.wait_ge(s_pe, 1)
            ve.tensor_copy(bc_sb[:], ps_bc[:]).then_inc(s_dv)          # dv=1
            ve.wait_ge(s_pe, 2)
            ve.tensor_copy(wfcT_sb[:], ps_w[:]).then_inc(s_dv)         # dv=2
            # -------- stage A --------
            for g in range(NGRP):
                nt = len(tiles_of(g))
                ve.wait_ge(s_pe, PE_Z[g])
                ve.tensor_copy(zTs[g % 2][:, :nt * 128],
                               ps_zs[g % 2][:, :nt * 128]).then_inc(s_dv)
                ve.wait_ge(s_pe, PE_TRL[g])
                if g >= 2:
                    ve.wait_ge(s_gp, 16 * (g // 2))  # zpack half flushed
                ve.tensor_copy(zpack[:, 64 * (g % 2):64 * (g % 2) + 16 * nt],
                               ps_tr4[:, :16 * nt]).then_inc(s_dv)
                ve.wait_ge(s_pe, PE_AL[g])
                ve.tensor_copy(t_all[:, GRP * g:GRP * g + nt],
                               ps_a4[:, :nt]).then_inc(s_dv)
            # -------- stage B: PRE(it) interleaved with POST(it-1) --------
            for it in range(TPC + 1):
                T, P = it, it - 1
                if T < TPC:
                    bT, b3T, KT = T % 2, T % 3, int(K_sched[T])
                    zgT, blT, zsT = zgs[b3T], blobs[b3T], zss[bT]
                    e4T = blT[:, 0:4 * KT]
                    qmT = blT[:, 4 * KT:8 * KT]
                    b01T = blT[:, 8 * KT:9 * KT]
                    aT = wks[bT][:, 0:KT]
                    xT = wks[bT][:, KMAX:KMAX + KT]
                    # u reuses the ae slot (ae dead after the x STT)
                    uT = wks[bT][:, 7 * KMAX:7 * KMAX + KT]
                    exvT = wks[bT][:, 3 * KMAX:3 * KMAX + 4 * KT]
                    exT = wks[bT][:, 8 * KMAX:8 * KMAX + 2 * KT]
                    aeT = wks[bT][:, 7 * KMAX:7 * KMAX + KT]
                    zgf = zgT[:, :64 * KT].rearrange("p (kq d) -> p kq d", d=16)
                    zgh = zgT[:, :64 * KT].rearrange("p (k two qd) -> p k two qd",
                                                     two=2, qd=32)
                    zgq = zgT[:, :64 * KT].rearrange("p (k q d) -> p k q d",
                                                     q=4, d=16)
                    zs3T = zsT[:, :16 * KT].rearrange("p (k d) -> p k d", d=16)
                    sc3T = sc16s[bT][:, :16 * KT].rearrange("p (k d) -> p k d", d=16)
                if P >= 0:
                    bP, KP = P % 2, int(K_sched[P])
                    zsP = zss[bP]
                    wvP = wks[bP][:, 2 * KMAX:2 * KMAX + KP]
                    exvP = wks[bP][:, 3 * KMAX:3 * KMAX + 4 * KP]
                    exP = wks[bP][:, 8 * KMAX:8 * KMAX + 2 * KP]
                    zs3P = zsP[:, :16 * KP].rearrange("p (k d) -> p k d", d=16)
                    sc3P = sc16s[bP][:, :16 * KP].rearrange("p (k d) -> p k d",
                                                            d=16)
                    denP, rdenP, wexP = (smals[bP][:, 0:1], smals[bP][:, 1:2],
                                         smals[bP][:, 2:4])

                # ---- L1 ----
                if T < TPC:
                    for q in range(4):
                        if qsnap[T][q] > 0:
                            ve.wait_ge(s_g[q], 16 * qsnap[T][q])
                    ve.wait_ge(s_tl[b3T], TL(T))
                    ve.tensor_tensor(out=zgf, in0=zgf,
                                     in1=qmT.to_broadcast([128, 4 * KT, 16]),
                                     op=OP.mult)
                if P >= 0:
                    ve.wait_ge(s_ac, P + 1)
                    ve.tensor_tensor(out=sc3P, in0=zs3P,
                                     in1=wvP.to_broadcast([128, KP, 16]),
                                     op=OP.mult)
                    ve.tensor_tensor(
                        out=exvP[:, 0:2 * KP].rearrange("p (k x) -> p k x", x=2),
                        in0=exP.rearrange("p (k x) -> p k x", x=2),
                        in1=wvP.to_broadcast([128, KP, 2]), op=OP.mult)
                ve.drain()
                # ---- L2 ----
                if T < TPC:
                    ve.tensor_tensor(out=zgh[:, :, 0, :], in0=zgh[:, :, 0, :],
                                     in1=zgh[:, :, 1, :], op=OP.add)
                if P >= 0:
                    ve.scalar_tensor_tensor(out=denP, in0=denP, scalar=1e-30,
                                            in1=zcol[:, 0:1], op0=OP.add,
                                            op1=OP.add)
                    ve.tensor_reduce(
                        out=obuf[:, 0:16],
                        in_=sc16s[bP][:, :16 * KP].rearrange(
                            "p (k d) -> p d k", d=16),
                        axis=AX.X, op=OP.add)
                if T < TPC:
                    ve.tensor_tensor(
                        out=exvT.rearrange("p (k x) -> p k x", x=4),
                        in0=e4T.rearrange("p (k x) -> p k x", x=4),
                        in1=bc_sb[:, C_W4:C_W4 + 4].to_broadcast([128, 4, KT])
                            .rearrange("p x k -> p k x"),
                        op=OP.mult)
                ve.drain()
                # ---- L3 ----
                if T < TPC:
                    ve.tensor_tensor(out=zs3T, in0=zgq[:, :, 0, :],
                                     in1=zgq[:, :, 1, :],
                                     op=OP.add).then_inc(s_dv)      # SEL[T]
                if P >= 0:
                    ve.tensor_reduce(
                        out=wexP,
                        in_=exvP[:, 0:2 * KP].rearrange("p (k x) -> p x k", x=2),
                        axis=AX.X, op=OP.add)
                    ve.reciprocal(rdenP, denP)
                if T < TPC:
                    ve.tensor_reduce(
                        out=exT.rearrange("p (k x) -> p k x", x=2),
                        in_=exvT.rearrange("p (k x two) -> p k x two", x=2, two=2),
                        axis=AX.X, op=OP.add)
                ve.drain()
                # ---- L4 ----
                if T < TPC:
                    ve.tensor_tensor(
                        out=sc3T, in0=zs3T,
                        in1=bc_sb[:, C_WAS:C_WAS + 16].to_broadcast([128, 16, KT])
                            .rearrange("p d k -> p k d"),
                        op=OP.mult)
                if P >= 0:
                    ve.scalar_tensor_tensor(
                        out=obuf[:, 16:32], in0=bc_sb[:, C_E2N0:C_E2N0 + 16],
                        scalar=wexP[:, 0:1], in1=obuf[:, 0:16],
                        op0=OP.mult, op1=OP.add)
                if T < TPC:
                    ve.tensor_tensor(
                        out=exvT[:, 0:2 * KT].rearrange("p (k x) -> p k x", x=2),
                        in0=exT.rearrange("p (k x) -> p k x", x=2),
                        in1=bc_sb[:, C_WAE:C_WAE + 2].to_broadcast([128, 2, KT])
                            .rearrange("p x k -> p k x"),
                        op=OP.mult)
                ve.drain()
                # ---- L5 ----
                if T < TPC:
                    ve.tensor_reduce(out=xT, in_=sc3T, axis=AX.X, op=OP.add)
                if P >= 0:
                    ve.scalar_tensor_tensor(
                        out=obuf[:, 0:16], in0=bc_sb[:, C_E2N1:C_E2N1 + 16],
                        scalar=wexP[:, 1:2], in1=obuf[:, 16:32],
                        op0=OP.mult, op1=OP.add)
                if T < TPC:
                    ve.tensor_reduce(
                        out=aeT,
                        in_=exvT[:, 0:2 * KT].rearrange("p (k x) -> p k x", x=2),
                        axis=AX.X, op=OP.add)
                ve.drain()
                # ---- L6 ----
                if T < TPC:
                    ve.scalar_tensor_tensor(
                        out=xT, in0=xT, scalar=t_all[:, T:T + 1], in1=aeT,
                        op0=OP.add, op1=OP.add)
                if P >= 0:
                    if P >= 2:
                        ve.wait_ge(s_ot[bP], 16 * (P // 2))
                    ve.scalar_tensor_tensor(
                        out=otiles[bP][:], in0=obuf[:, 0:16], scalar=rdenP,
                        in1=zcol[:].to_broadcast([128, 16]),
                        op0=OP.mult, op1=OP.add).then_inc(s_dv)     # OT[P]
                ve.drain()
                # ---- L7 ----
                if T < TPC:
                    ve.scalar_tensor_tensor(
                        out=aT, in0=b01T, scalar=100.0, in1=xT,
                        op0=OP.mult, op1=OP.add)
                    ve.scalar_tensor_tensor(
                        out=uT, in0=xT, scalar=0.01, in1=b01T,
                        op0=OP.mult, op1=OP.add)
                    ve.drain()
                    # ---- L8 ----
                    ve.tensor_tensor(out=aT, in0=aT, in1=uT,
                                     op=OP.max).then_inc(s_aw)

    nc.compile()
    return nc


_CACHE = {}


def kernel(h, e, src, dst, W_fc, W_attn, W_edge, W_e2n):
    import concourse.bass_utils as bu

    h = np.asarray(h, np.float32)
    e = np.asarray(e, np.float32)
    src = np.asarray(src, np.int64)
    dst = np.asarray(dst, np.int64)
    W_fc = np.asarray(W_fc, np.float32)
    W_attn = np.asarray(W_attn, np.float32)
    W_edge = np.asarray(W_edge, np.float32)
    W_e2n = np.asarray(W_e2n, np.float32)

    K_sched, col_off, idx16, blob, hsh, node_at = _host_prep(h, e, src, dst)

    key = tuple(K_sched.tolist())
    if key not in _CACHE:
        _CACHE[key] = _build(K_sched, col_off)
    nc = _CACHE[key]

    aux = np.zeros((1, 72), np.float32)
    aux[0, 0:2] = W_edge[0, :]
    aux[0, 2:4] = W_edge[1, :]
    aux[0, 4:6] = W_attn[0, 2 * OUT_DIM:]
    aux[0, 6:22] = W_attn[0, :OUT_DIM]
    aux[0, 22:38] = W_e2n[:, 0]
    aux[0, 38:54] = W_e2n[:, 1]
    auxc = W_attn[0, OUT_DIM:2 * OUT_DIM].reshape(OUT_DIM, 1).astype(np.float32)
    ident = np.eye(128, dtype=np.float32)
    ones = np.ones((1, 128), np.float32)
    zcol = np.zeros((128, 1), np.float32)

    in_maps = [{
        "hT": hsh[c], "wfc": W_fc, "aux": aux, "auxc": auxc,
        "ident": ident, "ones": ones, "zcol": zcol,
        "idx16": idx16[c], "blob": blob[c],
    } for c in range(N_CORES)]
    trace = bool(int(os.environ.get("AGAT_TRACE", "0")))
    if trace:
        _install_ntff_shim()
    res = bu.run_bass_kernel_spmd(nc, in_maps, core_ids=list(range(N_CORES)),
                                  trace=trace)
    global LAST_EXEC_NS
    LAST_EXEC_NS = res.exec_time_ns

    out = np.zeros((N_NODES, OUT_DIM), np.float32)
    for c in range(N_CORES):
        oc = res.results[c]["out"].reshape(BLOCK, OUT_DIM)
        rows = node_at[c]
        v = rows >= 0
        out[rows[v]] = oc[v]
    return out
